# revision 1
# baseline (speedup 1.0000x reference)
"""GCLayer (GNN message passing) on 8 Trainium2 NeuronCores.

Strategy
--------
Edges are partitioned by destination row (node-range sharding): core c owns
nodes [c*6250, (c+1)*6250) and every edge whose `row` lands there, so the
segment-sum needs no cross-core reduction.

Phase 1 (node stage, sharded): x = h@lin_w+b, x_ = msg_mlp(x),
A = x@att_w1[:D]+att_b1 per local node.  The host then assembles the global
x / x_ tables (bf16) and feeds them to every core ("replicate node features"
per the sharding hint; replication happens host-side between the two NEFFs).

Phase 2 (edge + out stage): per 128-edge tile, with edges sorted by row,
  hidden = A[row] + x[col]@Wc + ea*we   (PE: one-hot window matmul + gather-T
                                         matmul + rank-1, accumulated in PSUM)
  att    = sigmoid(silu(hidden)@w2 + b2 + maskbias)   (ACT + fused DVE reduce)
  msg[r] += att_e * x_[col_e]           (PE: sel-matmul into a PSUM window,
                                         sel[e,s] = att_e * (lrow_e == s))
then out = LN(x + out_mlp(msg)) per node block.

x[col] / x_[col] are fetched with SWDGE dma_gather (bf16, 256 B rows).  The
int16 index limit (<=32767) is handled by splitting each core's edge stream
into a "lo" pass (col < 32768) and a "hi" pass (col >= 32768) with statically
padded per-window blocks; both passes accumulate the same per-window PSUM
tile (lo: copy to SBUF accumulator, hi: add).
"""

import sys

sys.path.insert(0, "/opt/trn_rl_repo")

import numpy as np
import ml_dtypes

from concourse import bacc, mybir, tile
from concourse.bass_utils import run_bass_kernel_spmd

BF16 = ml_dtypes.bfloat16
F32 = np.float32

NCORES = 8
N = 50000
E = 800000
D = 128
NL = N // NCORES          # 6250 real nodes per core
NW = 49                   # node blocks of 128 per core (49*128 = 6272)
NLP = NW * 128            # padded nodes per core
SPLIT = 32768             # int16 gather index limit
CHUNK = 8192              # edges per dma_gather call
LN_EPS = 1e-5
MASK_BIAS = -30000.0      # sigmoid(x + MASK_BIAS) == 0 in fp32

FP = mybir.dt.float32
BF = mybir.dt.bfloat16
I16 = mybir.dt.int16

TRACE = False             # test.py sets kernel.TRACE = True for profiling
LAST_RESULTS = {}         # exec_time_ns per phase when TRACE

_cache = {}


def _bf(a):
    return np.ascontiguousarray(np.asarray(a, dtype=F32).astype(BF16))


def _f32(a):
    return np.ascontiguousarray(np.asarray(a, dtype=F32))


def _ceil(a, m):
    return -(-int(a) // m) * m


# ---------------------------------------------------------------------------
# Host-side edge preprocessing
# ---------------------------------------------------------------------------

def _prep_edges(row, col, ea, em, b2):
    """Sort/partition/pad edges. Returns per-core streams + the static plan."""
    row = np.asarray(row).astype(np.int64).ravel()
    col = np.asarray(col).astype(np.int64).ravel()
    ea = np.asarray(ea, dtype=F32).ravel()
    em = np.asarray(em, dtype=F32).ravel()

    c_of = row // NL
    r_loc = row - c_of * NL
    w_of = r_loc // 128
    s_in_w = r_loc % 128
    hi = (col >= SPLIT).astype(np.int64)

    # bucket = (core, pass(lo/hi), window); argsort gives the stream order
    key = (c_of * 2 + hi) * NW + w_of
    order = np.argsort(key, kind="stable")
    skey = key[order]
    cnt = np.bincount(key, minlength=NCORES * 2 * NW).reshape(NCORES, 2, NW)

    WL = np.array([_ceil(cnt[:, 0, w].max(), 128) for w in range(NW)])
    WH = np.array([_ceil(cnt[:, 1, w].max(), 128) for w in range(NW)])
    lo_total = int(WL.sum())
    hi_total = int(WH.sum())
    EP = lo_total + hi_total

    # padded base offset of each (pass, window) block within the stream
    sizes = np.concatenate([WL, WH])                      # (2*NW,)
    base = np.concatenate([[0], np.cumsum(sizes)[:-1]])   # (2*NW,)

    # rank of each edge within its bucket
    bstart = np.concatenate([[0], np.cumsum(cnt.ravel())[:-1]])
    rank = np.arange(row.size) - bstart[skey]
    pw = skey % (2 * NW)                                   # (pass, window) id
    dest = base[pw] + rank                                 # position in stream
    cc = skey // (2 * NW)

    g_lrow = np.zeros((NCORES, EP), F32)
    g_idx = np.zeros((NCORES, EP), np.int16)
    g_ea = np.zeros((NCORES, EP), F32)
    g_mb = np.full((NCORES, EP), b2 + MASK_BIAS, F32)      # pad edges -> att 0

    e_ids = order
    g_lrow[cc, dest] = s_in_w[e_ids].astype(F32)
    g_idx[cc, dest] = (col[e_ids] - hi[e_ids] * SPLIT).astype(np.int16)
    g_ea[cc, dest] = ea[e_ids]
    g_mb[cc, dest] = b2 + (em[e_ids] - 1.0) * (-MASK_BIAS) * (-1.0)

    # wrapped layouts
    lrow_col = np.ascontiguousarray(
        g_lrow.reshape(NCORES, EP // 128, 128).transpose(0, 2, 1))
    mb_col = np.ascontiguousarray(
        g_mb.reshape(NCORES, EP // 128, 128).transpose(0, 2, 1))
    idx16 = g_idx.reshape(NCORES, EP // 16, 16).transpose(0, 2, 1)  # (8,16,EP/16)
    idx_w = np.ascontiguousarray(np.tile(idx16, (1, 8, 1)))         # (8,128,EP/16)
    lrow_row = _bf(g_lrow.reshape(NCORES, 1, EP))
    ea_row = _bf(g_ea.reshape(NCORES, 1, EP))

    def chunks(total, start):
        out = []
        off = 0
        while off < total:
            g = min(CHUNK, total - off)
            out.append((start + off, g))
            off += g
        return out

    plan = dict(
        WL=[int(x) for x in WL], WH=[int(x) for x in WH], EP=EP,
        lo_total=lo_total, hi_total=hi_total,
        chunks_lo=chunks(lo_total, 0), chunks_hi=chunks(hi_total, lo_total),
    )
    streams = dict(lrow_col=lrow_col, mb_col=mb_col, idx=idx_w,
                   lrow_row=lrow_row, ea_row=ea_row)
    return plan, streams


# ---------------------------------------------------------------------------
# Phase 1: node stage (sharded over nodes)
# ---------------------------------------------------------------------------

def _build_phase1():
    nc = bacc.Bacc("TRN2", target_bir_lowering=False, debug=False,
                   num_devices=NCORES)
    g = lambda n, s, d, k: nc.dram_tensor(n, s, d, kind=k).ap()

    ht = g("ht", [128, NLP], BF, "ExternalInput")          # h^T, node-wrapped
    linw = g("linw", [128, 128], BF, "ExternalInput")
    linb = g("linb_row", [1, 128], BF, "ExternalInput")
    w1m = g("w1m", [128, 128], BF, "ExternalInput")
    b1m = g("b1m_row", [1, 128], BF, "ExternalInput")
    w2mp = g("w2mp", [128, 128], BF, "ExternalInput")
    b2mp = g("b2mp_row", [1, 128], BF, "ExternalInput")
    wr = g("wr", [128, 128], BF, "ExternalInput")
    b1a = g("b1a_row", [1, 128], BF, "ExternalInput")
    ident = g("ident", [128, 128], BF, "ExternalInput")
    ones1 = g("ones1", [1, 128], BF, "ExternalInput")
    eps_c = g("eps_col", [128, 1], FP, "ExternalInput")

    x_out = g("x_out", [128, NLP], FP, "ExternalOutput")
    xm_out = g("xm_out", [128, NLP], BF, "ExternalOutput")
    a_out = g("a_out", [128, NLP], BF, "ExternalOutput")

    with tile.TileContext(nc) as tc:
        with tc.tile_pool(name="const", bufs=1) as cp, \
             tc.tile_pool(name="work", bufs=3) as wp, \
             tc.tile_pool(name="psum", bufs=1, space="PSUM") as pp, \
             tc.tile_pool(name="psum2", bufs=2, space="PSUM") as pp2:

            def cload(ap, shape, dt, tag):
                t = cp.tile(shape, dt, tag=tag)
                nc.sync.dma_start(t[:], ap)
                return t

            ht_s = cload(ht, [128, NLP], BF, "c_ht")
            linw_s = cload(linw, [128, 128], BF, "c_linw")
            linb_s = cload(linb, [1, 128], BF, "c_linb")
            w1m_s = cload(w1m, [128, 128], BF, "c_w1m")
            b1m_s = cload(b1m, [1, 128], BF, "c_b1m")
            w2mp_s = cload(w2mp, [128, 128], BF, "c_w2mp")
            b2mp_s = cload(b2mp, [1, 128], BF, "c_b2mp")
            wr_s = cload(wr, [128, 128], BF, "c_wr")
            b1a_s = cload(b1a, [1, 128], BF, "c_b1a")
            id_s = cload(ident, [128, 128], BF, "c_id")
            ones_s = cload(ones1, [1, 128], BF, "c_ones")
            eps_s = cload(eps_c, [128, 1], FP, "c_eps")

            for w in range(NW):
                blk = slice(w * 128, (w + 1) * 128)
                htb = ht_s[:, blk]

                # x = h @ lin_w + lin_b      [n, d]
                px = pp.tile([128, 128], FP, tag="px")
                nc.tensor.matmul(px[:], htb, linw_s[:], start=True, stop=False)
                nc.tensor.matmul(px[:], ones_s[:], linb_s[:], start=False,
                                 stop=True)
                xs = wp.tile([128, 128], FP, tag="xs")
                nc.vector.tensor_copy(xs[:], px[:])
                nc.sync.dma_start(x_out[:, blk], xs[:])

                # x^T  (for downstream lhsT use)
                pxt = pp.tile([128, 128], FP, tag="pxt")
                nc.tensor.matmul(pxt[:], linw_s[:], htb, start=True, stop=False)
                nc.tensor.matmul(pxt[:], linb_s[:], ones_s[:], start=False,
                                 stop=True)
                xt = wp.tile([128, 128], BF, tag="xt")
                nc.vector.tensor_copy(xt[:], pxt[:])

                # A = x @ wr + b1a           [n, h]
                pa = pp.tile([128, 128], FP, tag="pa")
                nc.tensor.matmul(pa[:], xt[:], wr_s[:], start=True, stop=False)
                nc.tensor.matmul(pa[:], ones_s[:], b1a_s[:], start=False,
                                 stop=True)
                asb = wp.tile([128, 128], BF, tag="asb")
                nc.vector.tensor_copy(asb[:], pa[:])
                nc.sync.dma_start(a_out[:, blk], asb[:])

                # s1 = silu(x @ w1m + b1m)   [n, h]
                ps = pp.tile([128, 128], FP, tag="ps")
                nc.tensor.matmul(ps[:], xt[:], w1m_s[:], start=True, stop=False)
                nc.tensor.matmul(ps[:], ones_s[:], b1m_s[:], start=False,
                                 stop=True)
                s1 = wp.tile([128, 128], FP, tag="s1")
                nc.scalar.activation(s1[:], ps[:],
                                     mybir.ActivationFunctionType.Silu)

                # LayerNorm (affine folded into w2mp/b2mp)
                musum = wp.tile([128, 1], FP, tag="musum")
                nc.vector.reduce_sum(musum[:], s1[:],
                                     axis=mybir.AxisListType.X)
                mu = wp.tile([128, 1], FP, tag="mu")
                nc.scalar.activation(mu[:], musum[:],
                                     mybir.ActivationFunctionType.Copy,
                                     bias=0.0, scale=1.0 / 128.0)
                cen = wp.tile([128, 128], FP, tag="cen")
                nc.vector.tensor_scalar(cen[:], s1[:], mu[:, 0:1], None,
                                        mybir.AluOpType.subtract)
                sq = wp.tile([128, 128], FP, tag="sq")
                nc.scalar.square(sq[:], cen[:])
                vsum = wp.tile([128, 1], FP, tag="vsum")
                nc.vector.reduce_sum(vsum[:], sq[:],
                                     axis=mybir.AxisListType.X)
                std = wp.tile([128, 1], FP, tag="std")
                nc.scalar.activation(std[:], vsum[:],
                                     mybir.ActivationFunctionType.Sqrt,
                                     bias=eps_s[:, 0:1], scale=1.0 / 128.0)
                rstd = wp.tile([128, 1], FP, tag="rstd")
                nc.vector.reciprocal(rstd[:], std[:])
                z = wp.tile([128, 128], BF, tag="z")
                nc.vector.tensor_scalar(z[:], cen[:], rstd[:, 0:1], None,
                                        mybir.AluOpType.mult)

                # x_ = z @ w2mp + b2mp
                pzt = pp2.tile([128, 128], BF, tag="pzt")
                nc.tensor.transpose(pzt[:], z[:], id_s[:])
                zt = wp.tile([128, 128], BF, tag="zt")
                nc.vector.tensor_copy(zt[:], pzt[:])
                pxm = pp.tile([128, 128], FP, tag="pxm")
                nc.tensor.matmul(pxm[:], zt[:], w2mp_s[:], start=True,
                                 stop=False)
                nc.tensor.matmul(pxm[:], ones_s[:], b2mp_s[:], start=False,
                                 stop=True)
                xm = wp.tile([128, 128], BF, tag="xm")
                nc.vector.tensor_copy(xm[:], pxm[:])
                nc.sync.dma_start(xm_out[:, blk], xm[:])

    nc.compile()
    return nc


def _phase1_inputs(h, weights):
    """Per-core in_maps for phase 1."""
    (lin_w, lin_b, msg_w1, msg_b1, msg_ln_g, msg_ln_b, msg_w2, msg_b2,
     att_w1, att_b1) = weights
    w2mp = msg_ln_g[:, None] * msg_w2
    b2mp = msg_ln_b @ msg_w2 + msg_b2
    consts = {
        "linw": _bf(lin_w), "linb_row": _bf(lin_b[None, :]),
        "w1m": _bf(msg_w1), "b1m_row": _bf(msg_b1[None, :]),
        "w2mp": _bf(w2mp), "b2mp_row": _bf(b2mp[None, :]),
        "wr": _bf(att_w1[0:D]), "b1a_row": _bf(att_b1[None, :]),
        "ident": _bf(np.eye(128)), "ones1": _bf(np.ones((1, 128))),
        "eps_col": _f32(np.full((128, 1), LN_EPS)),
    }
    in_maps = []
    for c in range(NCORES):
        hc = np.zeros((NLP, D), F32)
        hc[:NL] = h[c * NL:(c + 1) * NL]
        in_maps.append({"ht": _bf(hc.T), **consts})
    return in_maps


def _run_phase1(h, weights):
    if "p1" not in _cache:
        _cache["p1"] = _build_phase1()
    nc = _cache["p1"]
    in_maps = _phase1_inputs(h, weights)
    res = run_bass_kernel_spmd(nc, in_maps, core_ids=list(range(NCORES)),
                               trace=TRACE)
    if TRACE:
        LAST_RESULTS["phase1_ns"] = res.exec_time_ns
    return res.results


# ---------------------------------------------------------------------------
# Phase 2: edge stage (gather/attention/scatter) + out stage
# ---------------------------------------------------------------------------

def _build_phase2(plan):
    WL, WH, EP = plan["WL"], plan["WH"], plan["EP"]
    chunks_lo, chunks_hi = plan["chunks_lo"], plan["chunks_hi"]
    lo_tiles = plan["lo_total"] // 128

    nc = bacc.Bacc("TRN2", target_bir_lowering=False, debug=False,
                   num_devices=NCORES)
    g = lambda n, s, d, k: nc.dram_tensor(n, s, d, kind=k).ap()

    xtab = g("xtab", [N, 128], BF, "ExternalInput")
    xmtab = g("xmtab", [N, 128], BF, "ExternalInput")
    aloc = g("aloc", [128, NLP], BF, "ExternalInput")
    xres = g("xres", [128, NLP], FP, "ExternalInput")
    lrowc = g("lrow_col", [128, EP // 128], FP, "ExternalInput")
    mbc = g("mb_col", [128, EP // 128], FP, "ExternalInput")
    idxt = g("idx", [128, EP // 16], I16, "ExternalInput")
    lrowr = g("lrow_row", [1, EP], BF, "ExternalInput")
    ear = g("ea_row", [1, EP], BF, "ExternalInput")

    iota_p = g("iota_part", [128, 128], FP, "ExternalInput")
    iota_f = g("iota_free", [128, 128], FP, "ExternalInput")
    wc = g("wc", [128, 128], BF, "ExternalInput")
    we = g("we_row", [1, 128], BF, "ExternalInput")
    w2r = g("w2rep", [128, 128], BF, "ExternalInput")
    ident = g("ident", [128, 128], BF, "ExternalInput")
    ones1 = g("ones1", [1, 128], BF, "ExternalInput")
    w1o = g("w1o", [128, 128], BF, "ExternalInput")
    b1o = g("b1o_row", [1, 128], BF, "ExternalInput")
    w2o = g("w2op", [128, 128], BF, "ExternalInput")
    b2o = g("b2op_row", [1, 128], BF, "ExternalInput")
    lngr = g("lng_rep", [128, 128], FP, "ExternalInput")
    lnbr = g("lnb_rep", [128, 128], FP, "ExternalInput")
    eps_c = g("eps_col", [128, 1], FP, "ExternalInput")

    out = g("out", [128, NLP], FP, "ExternalOutput")

    SILU = mybir.ActivationFunctionType.Silu
    SIGM = mybir.ActivationFunctionType.Sigmoid
    SQRT = mybir.ActivationFunctionType.Sqrt
    COPY = mybir.ActivationFunctionType.Copy
    EQ = mybir.AluOpType.is_equal
    MUL = mybir.AluOpType.mult
    ADD = mybir.AluOpType.add
    SUB = mybir.AluOpType.subtract
    AXX = mybir.AxisListType.X

    with tile.TileContext(nc) as tc:
        with tc.tile_pool(name="const", bufs=1) as cp, \
             tc.tile_pool(name="stream", bufs=1) as sp, \
             tc.tile_pool(name="acc", bufs=1) as accp, \
             tc.tile_pool(name="work", bufs=3) as wk:

            def cload(ap, shape, dt, tag, pool=None):
                t = (pool or cp).tile(shape, dt, tag=tag)
                nc.sync.dma_start(t[:], ap)
                return t

            aloc_s = cload(aloc, [128, NLP], BF, "c_aloc")
            xres_s = cload(xres, [128, NLP], FP, "c_xres")
            lrowc_s = cload(lrowc, [128, EP // 128], FP, "c_lrowc", sp)
            mb_s = cload(mbc, [128, EP // 128], FP, "c_mb", sp)
            idx_s = cload(idxt, [128, EP // 16], I16, "c_idx", sp)
            iop_s = cload(iota_p, [128, 128], FP, "c_iop")
            iof_s = cload(iota_f, [128, 128], FP, "c_iof")
            wc_s = cload(wc, [128, 128], BF, "c_wc")
            we_s = cload(we, [1, 128], BF, "c_we")
            w2r_s = cload(w2r, [128, 128], BF, "c_w2r")
            id_s = cload(ident, [128, 128], BF, "c_id")
            ones_s = cload(ones1, [1, 128], BF, "c_ones")
            w1o_s = cload(w1o, [128, 128], BF, "c_w1o")
            b1o_s = cload(b1o, [1, 128], BF, "c_b1o")
            w2o_s = cload(w2o, [128, 128], BF, "c_w2o")
            b2o_s = cload(b2o, [1, 128], BF, "c_b2o")
            lngr_s = cload(lngr, [128, 128], FP, "c_lngr")
            lnbr_s = cload(lnbr, [128, 128], FP, "c_lnbr")
            eps_s = cload(eps_c, [128, 1], FP, "c_eps")

            msg_acc = accp.tile([128, NLP], FP, tag="msg_acc")

            # ---- edge passes -------------------------------------------
            chp = tc.alloc_tile_pool(name="chunk", bufs=2)
            plp = tc.alloc_tile_pool(name="pl", bufs=2, space="PSUM")
            php = tc.alloc_tile_pool(name="ph", bufs=2, space="PSUM")
            pmp = tc.alloc_tile_pool(name="pm", bufs=2, space="PSUM")
            for pi, (wsizes, chunks, tbase, tab_lo) in enumerate([
                    (WL, chunks_lo, 0, True), (WH, chunks_hi, lo_tiles, False)]):
                if tab_lo:
                    xt_src, xm_src = xtab[0:SPLIT], xmtab[0:SPLIT]
                else:
                    xt_src, xm_src = xtab[SPLIT:N], xmtab[SPLIT:N]

                # chunk id for each in-pass tile index
                tile2chunk = []
                for ci, (off, gsz) in enumerate(chunks):
                    tile2chunk += [ci] * (gsz // 128)
                ck_tiles = {}
                rk_tiles = {}
                pass_start = chunks[0][0]
                pass_edges = sum(wsizes)
                RCH = 2048  # row-stream chunk (16 KB/partition if = CHUNK)

                j = 0
                for w in range(NW):
                    nt = wsizes[w] // 128
                    if nt == 0:
                        if tab_lo:
                            nc.vector.memset(
                                msg_acc[:, w * 128:(w + 1) * 128], 0.0)
                        continue
                    pm_t = pmp.tile([128, 128], FP, tag="pmsg")
                    for t in range(nt):
                        ci = tile2chunk[j]
                        off, gsz = chunks[ci]
                        # issue gathers at chunk starts
                        if j == (off - chunks[0][0]) // 128:
                            xt_c = chp.tile([128, 1, gsz], BF, tag="cxT")
                            nc.gpsimd.dma_gather(
                                xt_c[:], xt_src,
                                idx_s[:, off // 16:(off + gsz) // 16],
                                gsz, gsz, 128, transpose=True,
                                single_packet=False)
                            xm_c = chp.tile([128, gsz // 128, 128], BF,
                                            tag="cxm")
                            nc.gpsimd.dma_gather(
                                xm_c[:], xm_src,
                                idx_s[:, off // 16:(off + gsz) // 16],
                                gsz, gsz, 128, single_packet=False)
                            ck_tiles[ci] = (xt_c, xm_c)
                        rci = (j * 128) // RCH
                        if j * 128 == rci * RCH:
                            roff = pass_start + rci * RCH
                            rsz = min(RCH, pass_edges - rci * RCH)
                            lr_c = chp.tile([1, rsz], BF, tag="crow")
                            nc.sync.dma_start(lr_c[:], lrowr[0:1, roff:roff + rsz])
                            ea_c = chp.tile([1, rsz], BF, tag="cea")
                            nc.sync.dma_start(ea_c[:], ear[0:1, roff:roff + rsz])
                            rk_tiles[rci] = (lr_c, ea_c)
                        xt_c, xm_c = ck_tiles[ci]
                        lr_c, ea_c = rk_tiles[rci]
                        s = j - (off - chunks[0][0]) // 128
                        e0 = s * 128
                        e0r = j * 128 - rci * RCH
                        gt = tbase + j

                        prep = plp.tile([128, 128], FP, tag="plrep")
                        nc.tensor.matmul(prep[:], ones_s[:],
                                         lr_c[:, e0r:e0r + 128],
                                         start=True, stop=True)
                        oht = wk.tile([128, 128], BF, tag="oht")
                        nc.vector.tensor_tensor(oht[:], iop_s[:], prep[:], EQ)
                        ph_t = php.tile([128, 128], FP, tag="phid")
                        nc.tensor.matmul(ph_t[:], oht[:],
                                         aloc_s[:, w * 128:(w + 1) * 128],
                                         start=True, stop=False)
                        nc.tensor.matmul(ph_t[:], xt_c[:, 0, e0:e0 + 128],
                                         wc_s[:], start=False, stop=False)
                        nc.tensor.matmul(ph_t[:], ea_c[:, e0r:e0r + 128],
                                         we_s[:], start=False, stop=True)
                        sil = wk.tile([128, 128], BF, tag="sil")
                        nc.scalar.activation(sil[:], ph_t[:], SILU)
                        prod = wk.tile([128, 128], BF, tag="prod")
                        nc.vector.tensor_tensor(prod[:], sil[:], w2r_s[:], MUL)
                        logit = wk.tile([128, 1], FP, tag="logit")
                        nc.vector.reduce_sum(logit[:], prod[:], axis=AXX)
                        att = wk.tile([128, 1], FP, tag="att")
                        nc.scalar.activation(att[:], logit[:], SIGM,
                                             bias=mb_s[:, gt:gt + 1])
                        sel = wk.tile([128, 128], BF, tag="sel")
                        nc.vector.tensor_scalar(sel[:], iof_s[:],
                                                lrowc_s[:, gt:gt + 1],
                                                att[:, 0:1], EQ, MUL)
                        nc.tensor.matmul(pm_t[:], sel[:], xm_c[:, s, :],
                                         start=(t == 0), stop=(t == nt - 1))
                        j += 1
                    wblk = msg_acc[:, w * 128:(w + 1) * 128]
                    if tab_lo:
                        nc.vector.tensor_copy(wblk, pm_t[:])
                    else:
                        nc.vector.tensor_tensor(wblk, pm_t[:], wblk, ADD)

            for _p in (pmp, php, plp, chp):
                _p.release()

            # ---- out stage ---------------------------------------------
            ptp = tc.alloc_tile_pool(name="pt", bufs=2, space="PSUM")
            pop = tc.alloc_tile_pool(name="po", bufs=2, space="PSUM")
            for w in range(NW):
                wblk = slice(w * 128, (w + 1) * 128)
                mbf = wk.tile([128, 128], BF, tag="o_mbf")
                nc.vector.tensor_copy(mbf[:], msg_acc[:, wblk])
                ptr = ptp.tile([128, 128], BF, tag="o_ptr")
                nc.tensor.transpose(ptr[:], mbf[:], id_s[:])
                mt = wk.tile([128, 128], BF, tag="o_mt")
                nc.vector.tensor_copy(mt[:], ptr[:])
                po1 = pop.tile([128, 128], FP, tag="o_po1")
                nc.tensor.matmul(po1[:], mt[:], w1o_s[:], start=True,
                                 stop=False)
                nc.tensor.matmul(po1[:], ones_s[:], b1o_s[:], start=False,
                                 stop=True)
                s1o = wk.tile([128, 128], FP, tag="o_s1o")
                nc.scalar.activation(s1o[:], po1[:], SILU)

                musum = wk.tile([128, 1], FP, tag="o_mus")
                nc.vector.reduce_sum(musum[:], s1o[:], axis=AXX)
                mu = wk.tile([128, 1], FP, tag="o_mu")
                nc.scalar.activation(mu[:], musum[:], COPY, bias=0.0,
                                     scale=1.0 / 128.0)
                cen = wk.tile([128, 128], FP, tag="o_cen")
                nc.vector.tensor_scalar(cen[:], s1o[:], mu[:, 0:1], None, SUB)
                sq = wk.tile([128, 128], FP, tag="o_sq")
                nc.scalar.square(sq[:], cen[:])
                vsum = wk.tile([128, 1], FP, tag="o_vs")
                nc.vector.reduce_sum(vsum[:], sq[:], axis=AXX)
                std = wk.tile([128, 1], FP, tag="o_std")
                nc.scalar.activation(std[:], vsum[:], SQRT,
                                     bias=eps_s[:, 0:1], scale=1.0 / 128.0)
                rstd = wk.tile([128, 1], FP, tag="o_rstd")
                nc.vector.reciprocal(rstd[:], std[:])
                z = wk.tile([128, 128], BF, tag="o_z")
                nc.vector.tensor_scalar(z[:], cen[:], rstd[:, 0:1], None, MUL)
                pzt = ptp.tile([128, 128], BF, tag="o_pzt")
                nc.tensor.transpose(pzt[:], z[:], id_s[:])
                zt = wk.tile([128, 128], BF, tag="o_zt")
                nc.vector.tensor_copy(zt[:], pzt[:])
                po2 = pop.tile([128, 128], FP, tag="o_po2")
                nc.tensor.matmul(po2[:], zt[:], w2o_s[:], start=True,
                                 stop=False)
                nc.tensor.matmul(po2[:], ones_s[:], b2o_s[:], start=False,
                                 stop=True)

                r = wk.tile([128, 128], FP, tag="o_r")
                nc.vector.tensor_tensor(r[:], po2[:], xres_s[:, wblk], ADD)
                mus2 = wk.tile([128, 1], FP, tag="o_mus2")
                nc.vector.reduce_sum(mus2[:], r[:], axis=AXX)
                mu2 = wk.tile([128, 1], FP, tag="o_mu2")
                nc.scalar.activation(mu2[:], mus2[:], COPY, bias=0.0,
                                     scale=1.0 / 128.0)
                cen2 = wk.tile([128, 128], FP, tag="o_cen2")
                nc.vector.tensor_scalar(cen2[:], r[:], mu2[:, 0:1], None, SUB)
                sq2 = wk.tile([128, 128], FP, tag="o_sq2")
                nc.scalar.square(sq2[:], cen2[:])
                vs2 = wk.tile([128, 1], FP, tag="o_vs2")
                nc.vector.reduce_sum(vs2[:], sq2[:], axis=AXX)
                std2 = wk.tile([128, 1], FP, tag="o_std2")
                nc.scalar.activation(std2[:], vs2[:], SQRT,
                                     bias=eps_s[:, 0:1], scale=1.0 / 128.0)
                rstd2 = wk.tile([128, 1], FP, tag="o_rstd2")
                nc.vector.reciprocal(rstd2[:], std2[:])
                zf = wk.tile([128, 128], FP, tag="o_zf")
                nc.vector.tensor_scalar(zf[:], cen2[:], rstd2[:, 0:1], None,
                                        MUL)
                zg = wk.tile([128, 128], FP, tag="o_zg")
                nc.vector.tensor_tensor(zg[:], zf[:], lngr_s[:], MUL)
                ot = wk.tile([128, 128], FP, tag="o_ot")
                nc.vector.tensor_tensor(ot[:], zg[:], lnbr_s[:], ADD)
                nc.sync.dma_start(out[:, wblk], ot[:])
            pop.release()
            ptp.release()

    nc.compile()
    return nc


def _phase2_inputs(plan, streams, p1_results, inp):
    """Assemble per-core phase-2 in_maps from phase-1 outputs."""
    att_w1, att_b1 = inp["att_w1"], inp["att_b1"]
    att_w2, att_b2 = inp["att_w2"], inp["att_b2"]

    def unwrap(a, dt=F32):
        a = np.asarray(a, dtype=dt) if dt is not None else np.asarray(a)
        return a.reshape(128, NW, 128).transpose(1, 0, 2).reshape(NLP, 128)

    # global tables (bf16), padded to N rows only
    xtab = np.concatenate(
        [unwrap(p1_results[c]["x_out"])[:NL] for c in range(NCORES)])
    xmtab = np.concatenate(
        [unwrap(np.asarray(p1_results[c]["xm_out"], F32))[:NL]
         for c in range(NCORES)])
    xtab = _bf(xtab)
    xmtab = _bf(xmtab)

    w2op = inp["out_ln_g"][:, None] * inp["out_w2"]
    b2op = inp["out_ln_b"] @ inp["out_w2"] + inp["out_b2"]
    consts = {
        "xtab": xtab, "xmtab": xmtab,
        "iota_part": _f32(np.broadcast_to(np.arange(128)[:, None], (128, 128))),
        "iota_free": _f32(np.broadcast_to(np.arange(128)[None, :], (128, 128))),
        "wc": _bf(att_w1[D:2 * D]), "we_row": _bf(att_w1[2 * D:2 * D + 1]),
        "w2rep": _bf(np.broadcast_to(att_w2[:, 0][None, :], (128, 128))),
        "ident": _bf(np.eye(128)), "ones1": _bf(np.ones((1, 128))),
        "w1o": _bf(inp["out_w1"]), "b1o_row": _bf(inp["out_b1"][None, :]),
        "w2op": _bf(w2op), "b2op_row": _bf(b2op[None, :]),
        "lng_rep": _f32(np.broadcast_to(inp["ln_g"][None, :], (128, 128))),
        "lnb_rep": _f32(np.broadcast_to(inp["ln_b"][None, :], (128, 128))),
        "eps_col": _f32(np.full((128, 1), LN_EPS)),
    }
    in_maps = []
    for c in range(NCORES):
        in_maps.append({
            "aloc": np.ascontiguousarray(np.asarray(p1_results[c]["a_out"])),
            "xres": np.ascontiguousarray(np.asarray(p1_results[c]["x_out"])),
            "lrow_col": streams["lrow_col"][c],
            "mb_col": streams["mb_col"][c],
            "idx": streams["idx"][c],
            "lrow_row": streams["lrow_row"][c],
            "ea_row": streams["ea_row"][c],
            **consts,
        })
    return in_maps


def kernel(**inputs):
    inp = {k: np.asarray(v) for k, v in inputs.items()}
    h = _f32(inp["h"])
    weights = (inp["lin_w"], inp["lin_b"], inp["msg_w1"], inp["msg_b1"],
               inp["msg_ln_g"], inp["msg_ln_b"], inp["msg_w2"], inp["msg_b2"],
               inp["att_w1"], inp["att_b1"])
    b2 = float(np.asarray(inp["att_b2"]).ravel()[0])
    # edge_mask folded into the sigmoid bias (exact for 0/1 masks; the
    # problem's masks are all-ones)
    plan, streams = _prep_edges(inp["row"], inp["col"], inp["edge_attr"],
                                inp["edge_mask"], b2)

    p1 = _run_phase1(h, weights)

    key = (tuple(plan["WL"]), tuple(plan["WH"]))
    if _cache.get("p2_key") != key:
        _cache["p2"] = _build_phase2(plan)
        _cache["p2_key"] = key
    nc2 = _cache["p2"]
    in_maps = _phase2_inputs(plan, streams, p1, inp)
    res = run_bass_kernel_spmd(nc2, in_maps, core_ids=list(range(NCORES)),
                               trace=TRACE)
    if TRACE:
        LAST_RESULTS["phase2_ns"] = res.exec_time_ns
    out = np.concatenate([
        np.asarray(res.results[c]["out"], F32)
        .reshape(128, NW, 128).transpose(1, 0, 2).reshape(NLP, 128)[:NL]
        for c in range(NCORES)])
    return out.astype(F32)



# revision 45
# speedup vs baseline: 2.7966x; 2.7966x over previous
"""GCLayer (GNN message passing) on 8 Trainium2 NeuronCores — v2.

Strategy (see spec sharding_hint): edges partitioned by destination row;
node features replicated host-side between the two NEFFs.

Key points vs v1:
- ONE merged dma_gather per edge chunk fetching 512B rows [B[col] ; xm[col]]
  (B = x@wc precomputed in phase 1).  Q7 descriptor generation is the
  hard wall (~8 ns/desc), so descriptors are halved vs two 256B gathers.
- Node->(core,window,slot) assignment is load-balanced host-side so the
  static per-window tile counts carry ~2% padding instead of ~19%.
- Edge math is batched into 1024-edge macro-tiles: one DVE instruction per
  one-hot build / silu-product / logit-reduce; scatter via T-layout one-hot
  matmuls; silu runs directly on ACT with per-chunk table phases (no
  per-tile Silu<->Sigmoid table thrash).
- hi-pass windows run in reverse so per-window out-net work interleaves
  with the tail of the gather stream; LayerNorms run stage-major with
  exactly one Sqrt table load.
"""

import sys

sys.path.insert(0, "/opt/trn_rl_repo")

import numpy as np
import ml_dtypes

from concourse import bacc, mybir, tile
from concourse.bass_utils import run_bass_kernel_spmd

BF16 = ml_dtypes.bfloat16
F32 = np.float32

NCORES = 8
N = 50000
E = 800000
D = 128
NW = 49                   # windows (128 slots) per core
NLP = NW * 128            # 6272 slots per core
NBINS = NCORES * NW       # 392
SPLIT = 32768             # int16 gather index limit
CHUNK = 4096              # edges per dma_gather call == strip group
MACRO = 1024              # edges per DVE macro-tile
LN_EPS = 1e-5
MASK_BIAS = -30000.0

FP = mybir.dt.float32
BF = mybir.dt.bfloat16
I16 = mybir.dt.int16

SILU = mybir.ActivationFunctionType.Silu
IDEN = mybir.ActivationFunctionType.Identity
SIGM = mybir.ActivationFunctionType.Sigmoid
SQRT = mybir.ActivationFunctionType.Sqrt
COPY = mybir.ActivationFunctionType.Copy
EQ = mybir.AluOpType.is_equal
MUL = mybir.AluOpType.mult
ADD = mybir.AluOpType.add
SUB = mybir.AluOpType.subtract
AXX = mybir.AxisListType.X

TRACE = False
LAST_RESULTS = {}

_cache = {}


def _bf(a):
    return np.ascontiguousarray(np.asarray(a, dtype=F32).astype(BF16))


def _f32(a):
    return np.ascontiguousarray(np.asarray(a, dtype=F32))


def _ceil(a, m):
    return -(-int(a) // m) * m


# ---------------------------------------------------------------------------
# Host-side preprocessing
# ---------------------------------------------------------------------------

def _balance_nodes(row, col):
    """Assign nodes to (core, window, slot) balancing per-bin lo/hi edge
    counts.  Returns slot_node [NCORES, NLP] (node id or -1) and
    (c, w, s) arrays per node."""
    hi = col >= SPLIT
    d_lo = np.bincount(row[~hi], minlength=N).astype(np.int64)
    d_hi = np.bincount(row[hi], minlength=N).astype(np.int64)
    order = np.argsort(-(d_lo + d_hi), kind="stable")

    lo_sum = np.zeros(NBINS)
    hi_sum = np.zeros(NBINS)
    a_lo = max(d_lo.sum() / NBINS, 1.0)
    a_hi = max(d_hi.sum() / NBINS, 1.0)
    slot_node = np.full((NBINS, 128), -1, np.int64)
    node_bin = np.empty(N, np.int64)
    node_slot = np.empty(N, np.int64)
    for r in range(128):
        chunk = order[r * NBINS:(r + 1) * NBINS]
        if chunk.size == 0:
            break
        load = lo_sum / a_lo + hi_sum / a_hi
        bins = np.argsort(load, kind="stable")[:chunk.size]
        # heaviest nodes -> least-loaded bins (chunk already degree-sorted)
        slot_node[bins, r] = chunk
        node_bin[chunk] = bins
        node_slot[chunk] = r
        lo_sum[bins] += d_lo[chunk]
        hi_sum[bins] += d_hi[chunk]
    c_of = node_bin // NW
    w_of = node_bin % NW
    return slot_node.reshape(NCORES, NLP), c_of, w_of, node_slot


def _prep_edges(row, col, ea, em, b2):
    row = np.asarray(row).astype(np.int64).ravel()
    col = np.asarray(col).astype(np.int64).ravel()
    ea = np.asarray(ea, dtype=F32).ravel()
    em = np.asarray(em, dtype=F32).ravel()

    slot_node, c_of_n, w_of_n, s_of_n = _balance_nodes(row, col)
    c_of = c_of_n[row]
    w_of = w_of_n[row]
    s_in_w = s_of_n[row]
    hi = (col >= SPLIT).astype(np.int64)

    # processing order: lo pass windows ascending, hi pass windows descending
    worder = np.where(hi == 0, w_of, NW - 1 - w_of)
    key = (c_of * 2 + hi) * NW + worder
    order = np.argsort(key * (1 << 17) + col, kind="stable")
    skey = key[order]
    cnt = np.bincount(key, minlength=NCORES * 2 * NW).reshape(NCORES, 2, NW)

    # padded per-processing-position sizes (shared across cores)
    sizes_lo = np.array([_ceil(cnt[:, 0, j].max(), 128) for j in range(NW)])
    sizes_hi = np.array([_ceil(cnt[:, 1, j].max(), 128) for j in range(NW)])
    assert (sizes_lo > 0).all() and (sizes_hi > 0).all()
    sizes_lo[-1] += _ceil(sizes_lo.sum(), MACRO) - sizes_lo.sum()
    sizes_hi[-1] += _ceil(sizes_hi.sum(), MACRO) - sizes_hi.sum()
    EP_lo = int(sizes_lo.sum())
    EP_hi = int(sizes_hi.sum())
    EP = EP_lo + EP_hi

    sizes = np.concatenate([sizes_lo, sizes_hi])          # (2*NW,) proc order
    base = np.concatenate([[0], np.cumsum(sizes)[:-1]])

    bstart = np.concatenate([[0], np.cumsum(cnt.reshape(NCORES, -1)
                                            .reshape(-1))[:-1]])
    rank = np.arange(row.size) - bstart[skey]
    pw = skey % (2 * NW)
    dest = base[pw] + rank
    cc = skey // (2 * NW)

    g_lrow = np.zeros((NCORES, EP), F32)
    g_idx = np.zeros((NCORES, EP), np.int16)
    g_ea = np.zeros((NCORES, EP), F32)
    g_mb = np.full((NCORES, EP), b2 + MASK_BIAS, F32)

    e_ids = order
    g_lrow[cc, dest] = s_in_w[e_ids].astype(F32)
    g_idx[cc, dest] = (col[e_ids] - hi[e_ids] * SPLIT).astype(np.int16)
    g_ea[cc, dest] = ea[e_ids]
    g_mb[cc, dest] = b2 + (em[e_ids] - 1.0) * (-MASK_BIAS) * (-1.0)

    idx16 = g_idx.reshape(NCORES, EP // 16, 16).transpose(0, 2, 1)
    streams = dict(
        idx=np.ascontiguousarray(np.tile(idx16, (1, 8, 1))),
        lrow_row=_bf(g_lrow.reshape(NCORES, 1, EP)),
        lrowc=_bf(g_lrow.reshape(NCORES, EP // 128, 128).transpose(0, 2, 1)),
        mb=_f32(g_mb.reshape(NCORES, EP // 128, 128).transpose(0, 2, 1)),
        ea4=_bf(g_ea.reshape(NCORES, EP // 512, 4, 128)
                .transpose(0, 2, 1, 3).reshape(NCORES, 4, EP // 4)),
    )
    plan = dict(sizes_lo=tuple(int(x) for x in sizes_lo),
                sizes_hi=tuple(int(x) for x in sizes_hi))
    return plan, streams, slot_node


# ---------------------------------------------------------------------------
# Phase 1: node stage (same structure as v1 + B = x@wc output)
# ---------------------------------------------------------------------------

def _build_phase1():
    nc = bacc.Bacc("TRN2", target_bir_lowering=False, debug=False,
                   num_devices=NCORES)
    g = lambda n, s, d, k: nc.dram_tensor(n, s, d, kind=k).ap()

    ht = g("ht", [128, NLP], BF, "ExternalInput")
    linw = g("linw", [128, 128], BF, "ExternalInput")
    linb_r = g("linb_rep", [128, 128], FP, "ExternalInput")
    linb_c = g("linb_col", [128, 1], FP, "ExternalInput")
    w1m = g("w1m", [128, 128], BF, "ExternalInput")
    b1m_r = g("b1m_rep", [128, 128], FP, "ExternalInput")
    w2mp = g("w2mp", [128, 128], BF, "ExternalInput")
    b2mp_r = g("b2mp_rep", [128, 128], FP, "ExternalInput")
    wr = g("wr", [128, 128], BF, "ExternalInput")
    wc = g("wc", [128, 128], BF, "ExternalInput")
    b1a_r = g("b1a_rep", [128, 128], FP, "ExternalInput")
    ident = g("ident", [128, 128], BF, "ExternalInput")
    eps_c = g("eps_col", [128, 1], FP, "ExternalInput")

    x_out = g("x_out", [128, NLP], FP, "ExternalOutput")
    xm_out = g("xm_out", [128, NLP], BF, "ExternalOutput")
    a_out = g("a_out", [128, NLP], BF, "ExternalOutput")
    b_out = g("b_out", [128, NLP], BF, "ExternalOutput")

    TG = 7

    with tile.TileContext(nc) as tc:
        with tc.tile_pool(name="const", bufs=1) as cp, \
             tc.tile_pool(name="acc", bufs=1) as ac, \
             tc.tile_pool(name="work", bufs=3) as wp, \
             tc.tile_pool(name="psum", bufs=1, space="PSUM") as pp, \
             tc.tile_pool(name="psum2", bufs=2, space="PSUM") as pp2:

            def cload(ap, shape, dt, tag):
                t = cp.tile(shape, dt, tag=tag)
                nc.sync.dma_start(t[:], ap)
                return t

            ht_s = cload(ht, [128, NLP], BF, "c_ht")
            linw_s = cload(linw, [128, 128], BF, "c_linw")
            linbr_s = cload(linb_r, [128, 128], FP, "c_linbr")
            linbc_s = cload(linb_c, [128, 1], FP, "c_linbc")
            w1m_s = cload(w1m, [128, 128], BF, "c_w1m")
            b1mr_s = cload(b1m_r, [128, 128], FP, "c_b1mr")
            w2mp_s = cload(w2mp, [128, 128], BF, "c_w2mp")
            b2mpr_s = cload(b2mp_r, [128, 128], FP, "c_b2mpr")
            wr_s = cload(wr, [128, 128], BF, "c_wr")
            wc_s = cload(wc, [128, 128], BF, "c_wc")
            b1ar_s = cload(b1a_r, [128, 128], FP, "c_b1ar")
            id_s = cload(ident, [128, 128], BF, "c_id")
            eps_s = cload(eps_c, [128, 1], FP, "c_eps")

            s1_all = ac.tile([128, NW, 128], FP, tag="s1all")
            xt_all = ac.tile([128, NW, 128], BF, tag="xtall")
            z_all = ac.tile([128, NW, 128], BF, tag="zall")

            # ---- stage A: per-window matmuls ---------------------------
            for w in range(NW):
                blk = slice(w * 128, (w + 1) * 128)
                htb = ht_s[:, blk]

                px = pp.tile([128, 128], FP, tag="px")
                nc.tensor.matmul(px[:], htb, linw_s[:], start=True, stop=True)
                xs = wp.tile([128, 128], FP, tag="xs")
                nc.vector.tensor_tensor(xs[:], px[:], linbr_s[:], ADD)
                nc.sync.dma_start(x_out[:, blk], xs[:])

                pxt = pp.tile([128, 128], FP, tag="pxt")
                nc.tensor.matmul(pxt[:], linw_s[:], htb, start=True, stop=True)
                nc.scalar.activation(xt_all[:, w, :], pxt[:], IDEN,
                                     bias=linbc_s[:, 0:1])

                pa = pp.tile([128, 128], FP, tag="pa")
                nc.tensor.matmul(pa[:], xt_all[:, w, :], wr_s[:],
                                 start=True, stop=True)
                asb = wp.tile([128, 128], BF, tag="asb")
                nc.vector.tensor_tensor(asb[:], pa[:], b1ar_s[:], ADD)
                nc.sync.dma_start(a_out[:, blk], asb[:])

                pb = pp.tile([128, 128], FP, tag="pb")
                nc.tensor.matmul(pb[:], xt_all[:, w, :], wc_s[:],
                                 start=True, stop=True)
                bsb = wp.tile([128, 128], BF, tag="bsb")
                nc.vector.tensor_copy(bsb[:], pb[:])
                nc.sync.dma_start(b_out[:, blk], bsb[:])

                ps = pp.tile([128, 128], FP, tag="ps")
                nc.tensor.matmul(ps[:], xt_all[:, w, :], w1m_s[:],
                                 start=True, stop=True)
                nc.vector.tensor_tensor(s1_all[:, w, :], ps[:], b1mr_s[:], ADD)

            # ---- stage B: batched silu + LN (2 table loads total) ------
            var_all = ac.tile([128, NW], FP, tag="varall")
            cen_all = ac.tile([128, NW, 128], BF, tag="cenall")
            for g0 in range(0, NW, TG):
                nw7 = min(TG, NW - g0)
                src = s1_all[:, g0:g0 + nw7, :]
                sg = wp.tile([128, TG, 128], BF, tag="b_sg")
                nc.scalar.activation(sg[:, 0:nw7, :], src, SIGM)
                s1b = wp.tile([128, TG, 128], BF, tag="b_s1b")
                nc.vector.tensor_tensor(s1b[:, 0:nw7, :], src,
                                        sg[:, 0:nw7, :], MUL)
                muS = wp.tile([128, TG], FP, tag="b_muS")
                nc.vector.tensor_reduce(muS[:, 0:nw7], s1b[:, 0:nw7, :],
                                        axis=AXX, op=ADD)
                mu = wp.tile([128, TG], BF, tag="b_mu")
                nc.scalar.activation(mu[:, 0:nw7], muS[:, 0:nw7], COPY,
                                     scale=1.0 / 128.0)
                nc.vector.tensor_tensor(
                    cen_all[:, g0:g0 + nw7, :], s1b[:, 0:nw7, :],
                    mu[:, 0:nw7].unsqueeze(2).broadcast_to([128, nw7, 128]),
                    SUB)
                sq = wp.tile([128, TG, 128], BF, tag="b_sq")
                nc.vector.tensor_tensor(sq[:, 0:nw7, :],
                                        cen_all[:, g0:g0 + nw7, :],
                                        cen_all[:, g0:g0 + nw7, :], MUL)
                nc.vector.tensor_reduce(var_all[:, g0:g0 + nw7],
                                        sq[:, 0:nw7, :], axis=AXX, op=ADD)
            std_all = wp.tile([128, NW], FP, tag="b_std")
            nc.scalar.activation(std_all[:], var_all[:], SQRT,
                                 bias=eps_s[:, 0:1], scale=1.0 / 128.0)
            rstd_all = wp.tile([128, NW], BF, tag="b_rstd")
            with nc.allow_low_precision(reason="bf16 rstd is plenty"):
                nc.vector.reciprocal(rstd_all[:], std_all[:])
            for g0 in range(0, NW, TG):
                nw7 = min(TG, NW - g0)
                nc.vector.tensor_tensor(
                    z_all[:, g0:g0 + nw7, :], cen_all[:, g0:g0 + nw7, :],
                    rstd_all[:, g0:g0 + nw7].unsqueeze(2)
                    .broadcast_to([128, nw7, 128]), MUL)

            # ---- stage C: xm = z @ w2mp + b2mp -------------------------
            for w in range(NW):
                blk = slice(w * 128, (w + 1) * 128)
                pzt = pp2.tile([128, 128], BF, tag="pzt")
                nc.tensor.transpose(pzt[:], z_all[:, w, :], id_s[:])
                zt = wp.tile([128, 128], BF, tag="zt")
                nc.vector.tensor_copy(zt[:], pzt[:])
                pxm = pp.tile([128, 128], FP, tag="pxm")
                nc.tensor.matmul(pxm[:], zt[:], w2mp_s[:], start=True,
                                 stop=True)
                xm = wp.tile([128, 128], BF, tag="xm")
                nc.vector.tensor_tensor(xm[:], pxm[:], b2mpr_s[:], ADD)
                nc.sync.dma_start(xm_out[:, blk], xm[:])

    nc.compile()
    return nc


def _phase1_inputs(h_slots, inp):
    w2mp = inp["msg_ln_g"][:, None] * inp["msg_w2"]
    b2mp = inp["msg_ln_b"] @ inp["msg_w2"] + inp["msg_b2"]
    consts = {
        "linw": _bf(inp["lin_w"]),
        "linb_rep": _f32(np.broadcast_to(inp["lin_b"][None, :], (128, 128))),
        "linb_col": _f32(inp["lin_b"][:, None]),
        "w1m": _bf(inp["msg_w1"]),
        "b1m_rep": _f32(np.broadcast_to(inp["msg_b1"][None, :], (128, 128))),
        "w2mp": _bf(w2mp),
        "b2mp_rep": _f32(np.broadcast_to(b2mp[None, :], (128, 128))),
        "wr": _bf(inp["att_w1"][0:D]), "wc": _bf(inp["att_w1"][D:2 * D]),
        "b1a_rep": _f32(np.broadcast_to(inp["att_b1"][None, :], (128, 128))),
        "ident": _bf(np.eye(128)),
        "eps_col": _f32(np.full((128, 1), LN_EPS)),
    }
    in_maps = []
    for c in range(NCORES):
        in_maps.append({"ht": _bf(h_slots[c].T), **consts})
    return in_maps


def _run_phase1(h_slots, inp):
    if "p1" not in _cache:
        _cache["p1"] = _build_phase1()
    nc = _cache["p1"]
    res = run_bass_kernel_spmd(nc, _phase1_inputs(h_slots, inp),
                               core_ids=list(range(NCORES)), trace=TRACE)
    if TRACE:
        LAST_RESULTS["phase1_ns"] = res.exec_time_ns
        LAST_RESULTS["phase1_res"] = res
    return res.results


# ---------------------------------------------------------------------------
# Phase 2: edge stage + out stage
# ---------------------------------------------------------------------------

def _tile_table(sizes_lo, sizes_hi):
    """Per-tile (pass, window, first, last) in processing order."""
    tiles = []
    for p, sizes in ((0, sizes_lo), (1, sizes_hi)):
        for j, sz in enumerate(sizes):
            w = j if p == 0 else NW - 1 - j
            nt = sz // 128
            for t in range(nt):
                tiles.append((p, w, t == 0, t == nt - 1))
    return tiles


def _build_phase2(plan):
    sizes_lo, sizes_hi = plan["sizes_lo"], plan["sizes_hi"]
    EP_lo, EP_hi = sum(sizes_lo), sum(sizes_hi)
    EP = EP_lo + EP_hi
    tiles = _tile_table(sizes_lo, sizes_hi)
    NTILE = EP // 128
    NHI = N - SPLIT

    nc = bacc.Bacc("TRN2", target_bir_lowering=False, debug=False,
                   num_devices=NCORES)
    g = lambda n, s, d, k: nc.dram_tensor(n, s, d, kind=k).ap()

    tab_lo = g("tab_lo", [SPLIT, 256], BF, "ExternalInput")
    tab_hi = g("tab_hi", [NHI, 256], BF, "ExternalInput")
    aloc = g("aloc", [128, NLP], BF, "ExternalInput")
    xres = g("xres", [128, NLP], FP, "ExternalInput")
    idxt = g("idx", [128, EP // 16], I16, "ExternalInput")
    lrowr = g("lrow_row", [1, EP], BF, "ExternalInput")
    lrowc = g("lrowc", [128, EP // 128], BF, "ExternalInput")
    mbt = g("mb", [128, EP // 128], FP, "ExternalInput")
    ea4t = g("ea4", [4, EP // 4], BF, "ExternalInput")

    iof = g("iof", [128, 128], BF, "ExternalInput")
    iota_c = g("iota_col", [128, 1], FP, "ExternalInput")
    iota_cb = g("iota_colb", [128, 1], BF, "ExternalInput")
    ones1 = g("ones1", [1, 128], BF, "ExternalInput")
    ident = g("ident", [128, 128], BF, "ExternalInput")
    we4 = g("we4blk", [4, 512], BF, "ExternalInput")
    w2r = g("w2r", [128, 128], BF, "ExternalInput")
    w1o = g("w1o", [128, 128], BF, "ExternalInput")
    b1oc = g("b1o_col", [128, 1], FP, "ExternalInput")
    w2o = g("w2op", [128, 128], BF, "ExternalInput")
    b2o = g("b2op_row", [1, 128], BF, "ExternalInput")
    lngr = g("lng_rep", [128, 128], BF, "ExternalInput")
    lnbr = g("lnb_rep", [128, 128], BF, "ExternalInput")
    eps_c = g("eps_col", [128, 1], FP, "ExternalInput")

    out = g("out", [128, NLP], FP, "ExternalOutput")
    NTILE_ = EP // 128
    dbg_att = g("dbg_att", [128, NTILE_], BF, "ExternalOutput")
    dbg_msg = g("dbg_msg", [128, NLP], BF, "ExternalOutput")
    dbg_h1 = g("dbg_h1", [128, NLP], BF, "ExternalOutput")

    with tile.TileContext(nc) as tc:
        with tc.tile_pool(name="const", bufs=1) as cp, \
             tc.tile_pool(name="stream", bufs=1) as sp, \
             tc.tile_pool(name="acc", bufs=1) as accp, \
             tc.tile_pool(name="wk", bufs=2) as wk:

            def cload(ap, shape, dt, tag, pool=None):
                t = (pool or cp).tile(shape, dt, tag=tag)
                nc.sync.dma_start(t[:], ap)
                return t

            aloc_s = cload(aloc, [128, NLP], BF, "c_aloc")
            lrc_s = cload(lrowc, [128, EP // 128], BF, "c_lrc", sp)
            mb_s = cload(mbt, [128, EP // 128], FP, "c_mb", sp)
            iof_s = cload(iof, [128, 128], BF, "c_iof")
            ioc_s = cload(iota_c, [128, 1], FP, "c_ioc")
            iocb_s = cload(iota_cb, [128, 1], BF, "c_iocb")
            ones_s = cload(ones1, [1, 128], BF, "c_ones")
            id_s = cload(ident, [128, 128], BF, "c_id")
            we4_s = cload(we4, [4, 512], BF, "c_we4")
            w2r_s = cload(w2r, [128, 128], BF, "c_w2r")
            w1o_s = cload(w1o, [128, 128], BF, "c_w1o")
            b1o_s = cload(b1oc, [128, 1], FP, "c_b1o")
            w2o_s = cload(w2o, [128, 128], BF, "c_w2o")
            b2o_s = cload(b2o, [1, 128], BF, "c_b2o")
            lng_s = cload(lngr, [128, 128], BF, "c_lng")
            lnb_s = cload(lnbr, [128, 128], BF, "c_lnb")
            eps_s = cload(eps_c, [128, 1], FP, "c_eps")

            msgT = accp.tile([128, NW, 128], BF, tag="msgT")   # [d, w, slot]
            h1_all = accp.tile([128, NW, 128], BF, tag="h1all")

            chp = tc.alloc_tile_pool(name="chunk", bufs=4)
            idp = tc.alloc_tile_pool(name="idxp", bufs=4)
            selp = tc.alloc_tile_pool(name="selpool", bufs=8)
            ppp = tc.alloc_tile_pool(name="pprep", bufs=1, space="PSUM")
            php = tc.alloc_tile_pool(name="phid", bufs=2, space="PSUM")
            pmp = tc.alloc_tile_pool(name="pmsg", bufs=2, space="PSUM")
            pop = tc.alloc_tile_pool(name="pout", bufs=1, space="PSUM")

            # gather chunk boundaries (aligned to MACRO, within a pass)
            chunks = []
            for p, (p0, plen) in enumerate(((0, EP_lo), (EP_lo, EP_hi))):
                off = 0
                while off < plen:
                    gsz = min(CHUNK, plen - off)
                    chunks.append((p0 + off, gsz, p))
                    off += gsz

            ck_tiles = {}

            def issue_gather(ci):
                off, gsz, p = chunks[ci]
                src = tab_lo if p == 0 else tab_hi
                idxc = idp.tile([128, CHUNK // 16], I16, tag="idxc")
                nc.sync.dma_start(idxc[:, 0:gsz // 16],
                                  idxt[:, off // 16:(off + gsz) // 16])
                t = chp.tile([128, CHUNK // 128, 256], BF, tag="cx")
                nc.gpsimd.dma_gather(
                    t[:, 0:gsz // 128, :], src,
                    idxc[:, 0:gsz // 16],
                    gsz, gsz, 256, single_packet=False)
                ck_tiles[ci] = (t, off)

            for ci in range(min(3, len(chunks))):
                issue_gather(ci)

            msg_t = [None]
            closed_hi = []
            pending = [None]

            def part_a(ws):
                """out-net first linear + silu (sigmoid table) for windows ws
                (consecutive), into h1_all."""
                wmin = min(ws)
                nw = len(ws)
                p1 = pop.tile([128, 512], FP, tag="p1T")
                nc.tensor.matmul(p1[:, 0:nw * 128], w1o_s[:],
                                 msgT[:, wmin:wmin + nw, :],
                                 start=True, stop=True)
                sg = wk.tile([128, 512], BF, tag="a_sg")
                nc.scalar.activation(sg[:, 0:nw * 128], p1[:, 0:nw * 128],
                                     SIGM, bias=b1o_s[:, 0:1])
                ts_ = wk.tile([128, 512], BF, tag="a_ts")
                nc.scalar.activation(ts_[:, 0:nw * 128], p1[:, 0:nw * 128],
                                     IDEN, bias=b1o_s[:, 0:1])
                h1 = wk.tile([128, 512], BF, tag="a_h1")
                nc.vector.tensor_tensor(h1[:, 0:nw * 128], ts_[:, 0:nw * 128],
                                        sg[:, 0:nw * 128], MUL)
                for k, w in enumerate(sorted(ws)):
                    pt = pop.tile([128, 128], BF, tag="a_pt")
                    nc.tensor.transpose(pt[:], h1[:, k * 128:(k + 1) * 128],
                                        id_s[:])
                    nc.scalar.activation(h1_all[:, w, :], pt[:], COPY)

            def scatter_group(gstate):
                """Emit scatter matmuls + window closes for a finished chunk."""
                ci, sel_tiles = gstate
                ctile, coff = ck_tiles[ci]
                for mi, sel in sel_tiles:
                    for t in range(8):
                        gt = mi * 8 + t
                        p, w, first, last = tiles[gt]
                        loc = (gt * 128 - coff) // 128
                        if first:
                            msg_t[0] = pmp.tile([128, 128], FP, tag="msgw",
                                                name="msgw")
                        nc.tensor.matmul(msg_t[0][:],
                                         ctile[:, loc, 128:256],
                                         sel[:, t, :],
                                         start=first, stop=last)
                        if last:
                            if p == 0:
                                nc.scalar.activation(msgT[:, w, :],
                                                     msg_t[0][:], COPY)
                            else:
                                nc.vector.tensor_tensor(
                                    msgT[:, w, :], msg_t[0][:],
                                    msgT[:, w, :], ADD)
                                closed_hi.append(w)
                                if len(closed_hi) == 4 or w == 0:
                                    part_a(list(closed_hi))
                                    closed_hi.clear()

            for ci, (off, gsz, p) in enumerate(chunks):
                ctile, coff = ck_tiles[ci]
                # per-chunk row-side streams (full preload would blow SBUF)
                lrch = wk.tile([1, CHUNK], BF, tag="lrch")
                nc.sync.dma_start(lrch[0:1, 0:gsz], lrowr[0:1, off:off + gsz])
                each = wk.tile([4, CHUNK // 4], BF, tag="each")
                nc.sync.dma_start(each[:, 0:gsz // 4],
                                  ea4t[:, off // 4:(off + gsz) // 4])
                # scatter the PREVIOUS chunk first: its inputs are ready, so
                # the in-order PE does useful work while this chunk's gather
                # transfer completes
                if pending[0] is not None:
                    scatter_group(pending[0])
                    pending[0] = None
                gmacs = list(range(off // MACRO, (off + gsz) // MACRO))
                strip = wk.tile([128, 32], FP, tag="strip")
                for k, m in enumerate(gmacs):
                    moff = m * MACRO            # stream offset
                    # one-hot [s, e] for A-expand
                    prepb = wk.tile([128, 1024], BF, tag="prepb")
                    lmoff = moff - off
                    for h in range(2):
                        prep = ppp.tile([128, 512], FP, tag="prep")
                        nc.tensor.matmul(prep[:], ones_s[:],
                                         lrch[0:1, lmoff + h * 512:
                                              lmoff + (h + 1) * 512],
                                         start=True, stop=True)
                        nc.scalar.activation(prepb[:, h * 512:(h + 1) * 512],
                                             prep[:], COPY)
                    oht = wk.tile([128, 1024], BF, tag="oht")
                    nc.vector.tensor_tensor(
                        oht[:], prepb[:],
                        iocb_s[:, 0:1].broadcast_to([128, 1024]), EQ)
                    silu4 = wk.tile([128, 1024], BF, tag="silu4")
                    for h in range(2):
                        hid = php.tile([128, 512], FP, tag="hid")
                        # bank-wide opener first: start=True clears has_written
                        # for the WHOLE bank, so it must come before the
                        # per-region accumulating matmuls
                        m512 = lmoff // 512 + h
                        nc.tensor.matmul(hid[:],
                                         each[:, m512 * 128:(m512 + 1) * 128],
                                         we4_s[:], start=True, stop=False)
                        for t in range(4):
                            gt = m * 8 + h * 4 + t
                            w = tiles[gt][1]
                            nc.tensor.matmul(
                                hid[:, t * 128:(t + 1) * 128],
                                oht[:, (h * 4 + t) * 128:(h * 4 + t + 1) * 128],
                                aloc_s[:, w * 128:(w + 1) * 128],
                                start=False, stop=False)
                        loc0 = (moff - coff) // 128 + h * 4
                        nc.tensor.matmul(hid[:], id_s[:],
                                         ctile[:, loc0:loc0 + 4, 0:128],
                                         start=False, stop=True)
                        nc.scalar.activation(silu4[:, h * 512:(h + 1) * 512],
                                             hid[:], SILU)
                    sw = wk.tile([128, 8, 128], BF, tag="sw")
                    nc.vector.tensor_tensor(
                        sw[:],
                        silu4[:].rearrange("p (a b) -> p a b", b=128),
                        w2r_s[:].unsqueeze(1).broadcast_to([128, 8, 128]), MUL)
                    nc.vector.tensor_reduce(strip[:, k * 8:(k + 1) * 8],
                                            sw[:], axis=AXX, op=ADD)
                # strip -> att (one table flip per chunk)
                gt0 = gmacs[0] * 8
                ntl = len(gmacs) * 8
                stb = wk.tile([128, 32], FP, tag="stb")
                nc.vector.tensor_tensor(stb[:, 0:ntl], strip[:, 0:ntl],
                                        mb_s[:, gt0:gt0 + ntl], ADD)
                att = wk.tile([128, 32], BF, tag="att")
                nc.scalar.activation(att[:, 0:ntl], stb[:, 0:ntl], SIGM)
                nc.sync.dma_start(dbg_att[:, gt0:gt0 + ntl], att[:, 0:ntl])
                # sel one-hots (att folded) for each macro
                sel_tiles = []
                for k, m in enumerate(gmacs):
                    se = wk.tile([128, 8, 128], BF, tag="selEQ")
                    nc.vector.tensor_tensor(
                        se[:],
                        iof_s[:].unsqueeze(1).broadcast_to([128, 8, 128]),
                        lrc_s[:, m * 8:(m + 1) * 8].unsqueeze(2)
                        .broadcast_to([128, 8, 128]), EQ)
                    sa = selp.tile([128, 8, 128], BF, tag="selA")
                    nc.vector.tensor_tensor(
                        sa[:], se[:],
                        att[:, k * 8:(k + 1) * 8].unsqueeze(2)
                        .broadcast_to([128, 8, 128]), MUL)
                    sel_tiles.append((m, sa))
                pending[0] = (ci, sel_tiles)
                # prefetch AFTER the previous chunk's readers are emitted so
                # buffer reuse is dependency-tracked
                if ci + 3 < len(chunks):
                    issue_gather(ci + 3)
            scatter_group(pending[0])

            for _p in (pop, pmp, php, ppp, selp, idp, chp):
                _p.release()

            nc.sync.dma_start(dbg_msg, msgT[:])
            nc.sync.dma_start(dbg_h1, h1_all[:])

            # ---- tail: LayerNorms + second out-net linear + residual ------
            tp = tc.alloc_tile_pool(name="tailp", bufs=2, space="PSUM")
            tw = tc.alloc_tile_pool(name="tailw", bufs=2)
            TG = 7

            def ln_apply(src3, nw7, tagp):
                """LN over innermost 128 of src3 [128, nw7, 128] -> bf16 z."""
                muS = tw.tile([128, TG], FP, tag=f"{tagp}_muS")
                nc.vector.tensor_reduce(muS[:, 0:nw7], src3, axis=AXX, op=ADD)
                mu = tw.tile([128, TG], BF, tag=f"{tagp}_mu")
                nc.scalar.activation(mu[:, 0:nw7], muS[:, 0:nw7], COPY,
                                     scale=1.0 / 128.0)
                cen = tw.tile([128, TG, 128], BF, tag=f"{tagp}_cen")
                nc.vector.tensor_tensor(
                    cen[:, 0:nw7, :], src3,
                    mu[:, 0:nw7].unsqueeze(2).broadcast_to([128, nw7, 128]),
                    SUB)
                sq = tw.tile([128, TG, 128], BF, tag=f"{tagp}_sq")
                nc.vector.tensor_tensor(sq[:, 0:nw7, :], cen[:, 0:nw7, :],
                                        cen[:, 0:nw7, :], MUL)
                varS = tw.tile([128, TG], FP, tag=f"{tagp}_var")
                nc.vector.tensor_reduce(varS[:, 0:nw7], sq[:, 0:nw7, :],
                                        axis=AXX, op=ADD)
                std = tw.tile([128, TG], FP, tag=f"{tagp}_std")
                nc.scalar.activation(std[:, 0:nw7], varS[:, 0:nw7], SQRT,
                                     bias=eps_s[:, 0:1], scale=1.0 / 128.0)
                rstd = tw.tile([128, TG], BF, tag=f"{tagp}_rstd")
                with nc.allow_low_precision(reason="bf16 rstd is plenty"):
                    nc.vector.reciprocal(rstd[:, 0:nw7], std[:, 0:nw7])
                z = tw.tile([128, TG, 128], BF, tag=f"{tagp}_z")
                nc.vector.tensor_tensor(
                    z[:, 0:nw7, :], cen[:, 0:nw7, :],
                    rstd[:, 0:nw7].unsqueeze(2).broadcast_to([128, nw7, 128]),
                    MUL)
                return z

            for g0 in range(0, NW, TG):
                nw7 = min(TG, NW - g0)
                cs = slice(g0 * 128, (g0 + nw7) * 128)
                xr = tw.tile([128, TG * 128], FP, tag="t_xr")
                nc.sync.dma_start(xr[:, 0:nw7 * 128], xres[:, cs])

                z1 = ln_apply(h1_all[:, g0:g0 + nw7, :], nw7, "l1")
                r3 = tw.tile([128, TG, 128], BF, tag="t_r")
                for k in range(nw7):
                    pzt = tp.tile([128, 128], BF, tag="t_pzt")
                    nc.tensor.transpose(pzt[:], z1[:, k, :], id_s[:])
                    zt = tw.tile([128, 128], BF, tag="t_zt")
                    nc.scalar.activation(zt[:], pzt[:], COPY)
                    po2 = tp.tile([128, 128], FP, tag="t_po2")
                    nc.tensor.matmul(po2[:], zt[:], w2o_s[:], start=True,
                                     stop=False)
                    nc.tensor.matmul(po2[:], ones_s[:], b2o_s[:], start=False,
                                     stop=True)
                    nc.vector.tensor_tensor(
                        r3[:, k, :], po2[:],
                        xr[:, k * 128:(k + 1) * 128], ADD)
                z2 = ln_apply(r3[:, 0:nw7, :], nw7, "l2")
                zg = tw.tile([128, TG, 128], BF, tag="t_zg")
                nc.vector.tensor_tensor(
                    zg[:, 0:nw7, :], z2[:, 0:nw7, :],
                    lng_s[:].unsqueeze(1).broadcast_to([128, nw7, 128]), MUL)
                og = tw.tile([128, TG, 128], FP, tag="t_og")
                nc.vector.tensor_tensor(
                    og[:, 0:nw7, :], zg[:, 0:nw7, :],
                    lnb_s[:].unsqueeze(1).broadcast_to([128, nw7, 128]), ADD)
                nc.sync.dma_start(out[:, cs], og[:, 0:nw7, :])
            tw.release()
            tp.release()

    nc.compile()
    return nc


def _phase2_inputs(plan, streams, p1_results, inp, slot_node):
    att_w1, att_w2 = inp["att_w1"], inp["att_w2"]

    def unwrap(a, c):
        a = np.asarray(a, dtype=F32)
        return a.reshape(128, NW, 128).transpose(1, 0, 2).reshape(NLP, 128)

    # merged [B ; xm] table in original node order
    tab = np.zeros((N, 256), F32)
    for c in range(NCORES):
        bl = unwrap(p1_results[c]["b_out"], c)
        xl = unwrap(p1_results[c]["xm_out"], c)
        sn = slot_node[c]
        m = sn >= 0
        tab[sn[m], 0:128] = bl[m]
        tab[sn[m], 128:256] = xl[m]
    tab = _bf(tab)

    w2op = inp["out_ln_g"][:, None] * inp["out_w2"]
    b2op = inp["out_ln_b"] @ inp["out_w2"] + inp["out_b2"]
    we = att_w1[2 * D:2 * D + 1].ravel()       # [128]
    we4 = np.zeros((4, 512), F32)
    for k in range(4):
        we4[k, k * 128:(k + 1) * 128] = we

    consts = {
        "tab_lo": tab[0:SPLIT], "tab_hi": tab[SPLIT:N],
        "iof": _bf(np.broadcast_to(np.arange(128)[None, :], (128, 128))),
        "iota_col": _f32(np.arange(128)[:, None]),
        "iota_colb": _bf(np.arange(128)[:, None]),
        "ones1": _bf(np.ones((1, 128))), "ident": _bf(np.eye(128)),
        "we4blk": _bf(we4),
        "w2r": _bf(np.broadcast_to(att_w2[:, 0][None, :], (128, 128))),
        "w1o": _bf(inp["out_w1"]), "b1o_col": _f32(inp["out_b1"][:, None]),
        "w2op": _bf(w2op), "b2op_row": _bf(b2op[None, :]),
        "lng_rep": _bf(np.broadcast_to(inp["ln_g"][None, :], (128, 128))),
        "lnb_rep": _bf(np.broadcast_to(inp["ln_b"][None, :], (128, 128))),
        "eps_col": _f32(np.full((128, 1), LN_EPS)),
    }
    in_maps = []
    for c in range(NCORES):
        in_maps.append({
            "aloc": np.ascontiguousarray(np.asarray(p1_results[c]["a_out"])),
            "xres": np.ascontiguousarray(np.asarray(p1_results[c]["x_out"])),
            "idx": streams["idx"][c],
            "lrow_row": streams["lrow_row"][c],
            "lrowc": streams["lrowc"][c],
            "mb": streams["mb"][c],
            "ea4": streams["ea4"][c],
            **consts,
        })
    return in_maps


def kernel(**inputs):
    inp = {k: np.asarray(v) for k, v in inputs.items()}
    b2 = float(np.asarray(inp["att_b2"]).ravel()[0])
    plan, streams, slot_node = _prep_edges(
        inp["row"], inp["col"], inp["edge_attr"], inp["edge_mask"], b2)

    # phase-1 input: h gathered into (core, slot) layout
    h = _f32(inp["h"])
    h_slots = np.zeros((NCORES, NLP, D), F32)
    for c in range(NCORES):
        sn = slot_node[c]
        m = sn >= 0
        h_slots[c][m] = h[sn[m]]

    p1 = _run_phase1(h_slots, inp)

    key = (plan["sizes_lo"], plan["sizes_hi"])
    if _cache.get("p2_key") != key:
        _cache["p2"] = _build_phase2(plan)
        _cache["p2_key"] = key
    nc2 = _cache["p2"]
    in_maps = _phase2_inputs(plan, streams, p1, inp, slot_node)
    res = run_bass_kernel_spmd(nc2, in_maps, core_ids=list(range(NCORES)),
                               trace=TRACE)
    LAST_RESULTS["phase2_results"] = res.results
    if TRACE:
        LAST_RESULTS["phase2_ns"] = res.exec_time_ns
        LAST_RESULTS["phase2_res"] = res

    out = np.zeros((N, D), F32)
    for c in range(NCORES):
        o = np.asarray(res.results[c]["out"], F32)
        o = o.reshape(128, NW, 128).transpose(1, 0, 2).reshape(NLP, 128)
        sn = slot_node[c]
        m = sn >= 0
        out[sn[m]] = o[m]
    return out.astype(F32)


# revision 53
# speedup vs baseline: 2.8469x; 1.0180x over previous
"""GCLayer (GNN message passing) on 8 Trainium2 NeuronCores — v2.

Strategy (see spec sharding_hint): edges partitioned by destination row;
node features replicated host-side between the two NEFFs.

Key points vs v1:
- ONE merged dma_gather per edge chunk fetching 512B rows [B[col] ; xm[col]]
  (B = x@wc precomputed in phase 1).  Q7 descriptor generation is the
  hard wall (~8 ns/desc), so descriptors are halved vs two 256B gathers.
- Node->(core,window,slot) assignment is load-balanced host-side so the
  static per-window tile counts carry ~2% padding instead of ~19%.
- Edge math is batched into 1024-edge macro-tiles: one DVE instruction per
  one-hot build / silu-product / logit-reduce; scatter via T-layout one-hot
  matmuls; silu runs directly on ACT with per-chunk table phases (no
  per-tile Silu<->Sigmoid table thrash).
- hi-pass windows run in reverse so per-window out-net work interleaves
  with the tail of the gather stream; LayerNorms run stage-major with
  exactly one Sqrt table load.
"""

import sys

sys.path.insert(0, "/opt/trn_rl_repo")

import numpy as np
import ml_dtypes

from concourse import bacc, mybir, tile
from concourse.bass_utils import run_bass_kernel_spmd

BF16 = ml_dtypes.bfloat16
F32 = np.float32

NCORES = 8
N = 50000
E = 800000
D = 128
NW = 49                   # windows (128 slots) per core
NLP = NW * 128            # 6272 slots per core
NBINS = NCORES * NW       # 392
SPLIT = 32768             # int16 gather index limit
CHUNK = 4096              # edges per dma_gather call == strip group
MACRO = 1024              # edges per DVE macro-tile
LN_EPS = 1e-5
MASK_BIAS = -30000.0

FP = mybir.dt.float32
BF = mybir.dt.bfloat16
I16 = mybir.dt.int16

SILU = mybir.ActivationFunctionType.Silu
IDEN = mybir.ActivationFunctionType.Identity
SIGM = mybir.ActivationFunctionType.Sigmoid
SQRT = mybir.ActivationFunctionType.Sqrt
COPY = mybir.ActivationFunctionType.Copy
EQ = mybir.AluOpType.is_equal
MUL = mybir.AluOpType.mult
ADD = mybir.AluOpType.add
SUB = mybir.AluOpType.subtract
AXX = mybir.AxisListType.X

TRACE = False
LAST_RESULTS = {}

_cache = {}


def _bf(a):
    return np.ascontiguousarray(np.asarray(a, dtype=F32).astype(BF16))


def _f32(a):
    return np.ascontiguousarray(np.asarray(a, dtype=F32))


def _ceil(a, m):
    return -(-int(a) // m) * m


# ---------------------------------------------------------------------------
# Host-side preprocessing
# ---------------------------------------------------------------------------

def _balance_nodes(row, col):
    """Assign nodes to (core, window, slot) balancing per-bin lo/hi edge
    counts.  Returns slot_node [NCORES, NLP] (node id or -1) and
    (c, w, s) arrays per node."""
    hi = col >= SPLIT
    d_lo = np.bincount(row[~hi], minlength=N).astype(np.int64)
    d_hi = np.bincount(row[hi], minlength=N).astype(np.int64)
    order = np.argsort(-(d_lo + d_hi), kind="stable")

    lo_sum = np.zeros(NBINS)
    hi_sum = np.zeros(NBINS)
    a_lo = max(d_lo.sum() / NBINS, 1.0)
    a_hi = max(d_hi.sum() / NBINS, 1.0)
    slot_node = np.full((NBINS, 128), -1, np.int64)
    node_bin = np.empty(N, np.int64)
    node_slot = np.empty(N, np.int64)
    for r in range(128):
        chunk = order[r * NBINS:(r + 1) * NBINS]
        if chunk.size == 0:
            break
        load = lo_sum / a_lo + hi_sum / a_hi
        bins = np.argsort(load, kind="stable")[:chunk.size]
        # heaviest nodes -> least-loaded bins (chunk already degree-sorted)
        slot_node[bins, r] = chunk
        node_bin[chunk] = bins
        node_slot[chunk] = r
        lo_sum[bins] += d_lo[chunk]
        hi_sum[bins] += d_hi[chunk]
    c_of = node_bin // NW
    w_of = node_bin % NW
    return slot_node.reshape(NCORES, NLP), c_of, w_of, node_slot


def _prep_edges(row, col, ea, em, b2):
    row = np.asarray(row).astype(np.int64).ravel()
    col = np.asarray(col).astype(np.int64).ravel()
    ea = np.asarray(ea, dtype=F32).ravel()
    em = np.asarray(em, dtype=F32).ravel()

    slot_node, c_of_n, w_of_n, s_of_n = _balance_nodes(row, col)
    c_of = c_of_n[row]
    w_of = w_of_n[row]
    s_in_w = s_of_n[row]
    hi = (col >= SPLIT).astype(np.int64)

    # processing order: lo pass windows ascending, hi pass windows descending
    worder = np.where(hi == 0, w_of, NW - 1 - w_of)
    key = (c_of * 2 + hi) * NW + worder
    order = np.argsort(key * (1 << 17) + col, kind="stable")
    skey = key[order]
    cnt = np.bincount(key, minlength=NCORES * 2 * NW).reshape(NCORES, 2, NW)

    # padded per-processing-position sizes (shared across cores)
    sizes_lo = np.array([_ceil(cnt[:, 0, j].max(), 128) for j in range(NW)])
    sizes_hi = np.array([_ceil(cnt[:, 1, j].max(), 128) for j in range(NW)])
    assert (sizes_lo > 0).all() and (sizes_hi > 0).all()
    sizes_lo[-1] += _ceil(sizes_lo.sum(), MACRO) - sizes_lo.sum()
    sizes_hi[-1] += _ceil(sizes_hi.sum(), MACRO) - sizes_hi.sum()
    EP_lo = int(sizes_lo.sum())
    EP_hi = int(sizes_hi.sum())
    EP = EP_lo + EP_hi

    sizes = np.concatenate([sizes_lo, sizes_hi])          # (2*NW,) proc order
    base = np.concatenate([[0], np.cumsum(sizes)[:-1]])

    bstart = np.concatenate([[0], np.cumsum(cnt.reshape(NCORES, -1)
                                            .reshape(-1))[:-1]])
    rank = np.arange(row.size) - bstart[skey]
    pw = skey % (2 * NW)
    dest = base[pw] + rank
    cc = skey // (2 * NW)

    g_lrow = np.zeros((NCORES, EP), F32)
    g_idx = np.zeros((NCORES, EP), np.int16)
    g_ea = np.zeros((NCORES, EP), F32)
    g_mb = np.full((NCORES, EP), b2 + MASK_BIAS, F32)

    e_ids = order
    g_lrow[cc, dest] = s_in_w[e_ids].astype(F32)
    g_idx[cc, dest] = (col[e_ids] - hi[e_ids] * SPLIT).astype(np.int16)
    g_ea[cc, dest] = ea[e_ids]
    g_mb[cc, dest] = b2 + (em[e_ids] - 1.0) * (-MASK_BIAS) * (-1.0)

    idx16 = g_idx.reshape(NCORES, EP // 16, 16).transpose(0, 2, 1)
    streams = dict(
        idx=np.ascontiguousarray(np.tile(idx16, (1, 8, 1))),
        lrow_row=_bf(g_lrow.reshape(NCORES, 1, EP)),
        lrowc=_bf(g_lrow.reshape(NCORES, EP // 128, 128).transpose(0, 2, 1)),
        mb=_f32(g_mb.reshape(NCORES, EP // 128, 128).transpose(0, 2, 1)),
        ea4=_bf(g_ea.reshape(NCORES, EP // 512, 4, 128)
                .transpose(0, 2, 1, 3).reshape(NCORES, 4, EP // 4)),
    )
    plan = dict(sizes_lo=tuple(int(x) for x in sizes_lo),
                sizes_hi=tuple(int(x) for x in sizes_hi))
    return plan, streams, slot_node


# ---------------------------------------------------------------------------
# Phase 1: node stage (same structure as v1 + B = x@wc output)
# ---------------------------------------------------------------------------

def _build_phase1():
    nc = bacc.Bacc("TRN2", target_bir_lowering=False, debug=False,
                   num_devices=NCORES)
    g = lambda n, s, d, k: nc.dram_tensor(n, s, d, kind=k).ap()

    ht = g("ht", [128, NLP], BF, "ExternalInput")
    linw = g("linw", [128, 128], BF, "ExternalInput")
    linb_r = g("linb_rep", [128, 128], FP, "ExternalInput")
    linb_c = g("linb_col", [128, 1], FP, "ExternalInput")
    w1m = g("w1m", [128, 128], BF, "ExternalInput")
    b1m_r = g("b1m_rep", [128, 128], FP, "ExternalInput")
    w2mp = g("w2mp", [128, 128], BF, "ExternalInput")
    b2mp_r = g("b2mp_rep", [128, 128], FP, "ExternalInput")
    wr = g("wr", [128, 128], BF, "ExternalInput")
    wc = g("wc", [128, 128], BF, "ExternalInput")
    b1a_r = g("b1a_rep", [128, 128], FP, "ExternalInput")
    ident = g("ident", [128, 128], BF, "ExternalInput")
    eps_c = g("eps_col", [128, 1], FP, "ExternalInput")

    x_out = g("x_out", [128, NLP], FP, "ExternalOutput")
    xm_out = g("xm_out", [128, NLP], BF, "ExternalOutput")
    a_out = g("a_out", [128, NLP], BF, "ExternalOutput")
    b_out = g("b_out", [128, NLP], BF, "ExternalOutput")

    TG = 7

    with tile.TileContext(nc) as tc:
        with tc.tile_pool(name="const", bufs=1) as cp, \
             tc.tile_pool(name="acc", bufs=1) as ac, \
             tc.tile_pool(name="work", bufs=3) as wp, \
             tc.tile_pool(name="psum", bufs=1, space="PSUM") as pp, \
             tc.tile_pool(name="psum2", bufs=2, space="PSUM") as pp2:

            def cload(ap, shape, dt, tag):
                t = cp.tile(shape, dt, tag=tag)
                nc.sync.dma_start(t[:], ap)
                return t

            ht_s = cload(ht, [128, NLP], BF, "c_ht")
            linw_s = cload(linw, [128, 128], BF, "c_linw")
            linbr_s = cload(linb_r, [128, 128], FP, "c_linbr")
            linbc_s = cload(linb_c, [128, 1], FP, "c_linbc")
            w1m_s = cload(w1m, [128, 128], BF, "c_w1m")
            b1mr_s = cload(b1m_r, [128, 128], FP, "c_b1mr")
            w2mp_s = cload(w2mp, [128, 128], BF, "c_w2mp")
            b2mpr_s = cload(b2mp_r, [128, 128], FP, "c_b2mpr")
            wr_s = cload(wr, [128, 128], BF, "c_wr")
            wc_s = cload(wc, [128, 128], BF, "c_wc")
            b1ar_s = cload(b1a_r, [128, 128], FP, "c_b1ar")
            id_s = cload(ident, [128, 128], BF, "c_id")
            eps_s = cload(eps_c, [128, 1], FP, "c_eps")

            s1_all = ac.tile([128, NW, 128], FP, tag="s1all")
            xt_all = ac.tile([128, NW, 128], BF, tag="xtall")
            z_all = ac.tile([128, NW, 128], BF, tag="zall")
            x_acc = ac.tile([128, NW, 128], FP, tag="xacc")
            a_acc = ac.tile([128, NW, 128], BF, tag="aacc")
            b_acc = ac.tile([128, NW, 128], BF, tag="bacc")
            xm_acc = ac.tile([128, NW, 128], BF, tag="xmacc")

            # ---- stage A: per-window matmuls ---------------------------
            for w in range(NW):
                blk = slice(w * 128, (w + 1) * 128)
                htb = ht_s[:, blk]

                px = pp.tile([128, 128], FP, tag="px")
                nc.tensor.matmul(px[:], htb, linw_s[:], start=True, stop=True)
                nc.vector.tensor_tensor(x_acc[:, w, :], px[:], linbr_s[:], ADD)

                pxt = pp.tile([128, 128], FP, tag="pxt")
                nc.tensor.matmul(pxt[:], linw_s[:], htb, start=True, stop=True)
                nc.scalar.activation(xt_all[:, w, :], pxt[:], IDEN,
                                     bias=linbc_s[:, 0:1])

                pa = pp.tile([128, 128], FP, tag="pa")
                nc.tensor.matmul(pa[:], xt_all[:, w, :], wr_s[:],
                                 start=True, stop=True)
                nc.vector.tensor_tensor(a_acc[:, w, :], pa[:], b1ar_s[:], ADD)

                pb = pp.tile([128, 128], FP, tag="pb")
                nc.tensor.matmul(pb[:], xt_all[:, w, :], wc_s[:],
                                 start=True, stop=True)
                nc.vector.tensor_copy(b_acc[:, w, :], pb[:])

                ps = pp.tile([128, 128], FP, tag="ps")
                nc.tensor.matmul(ps[:], xt_all[:, w, :], w1m_s[:],
                                 start=True, stop=True)
                nc.vector.tensor_tensor(s1_all[:, w, :], ps[:], b1mr_s[:], ADD)
            nc.sync.dma_start(x_out, x_acc[:])
            nc.sync.dma_start(a_out, a_acc[:])
            nc.sync.dma_start(b_out, b_acc[:])

            # ---- stage B: batched silu + LN (2 table loads total) ------
            var_all = ac.tile([128, NW], FP, tag="varall")
            cen_all = ac.tile([128, NW, 128], BF, tag="cenall")
            for g0 in range(0, NW, TG):
                nw7 = min(TG, NW - g0)
                src = s1_all[:, g0:g0 + nw7, :]
                sg = wp.tile([128, TG, 128], BF, tag="b_sg")
                nc.scalar.activation(sg[:, 0:nw7, :], src, SIGM)
                s1b = wp.tile([128, TG, 128], BF, tag="b_s1b")
                nc.vector.tensor_tensor(s1b[:, 0:nw7, :], src,
                                        sg[:, 0:nw7, :], MUL)
                muS = wp.tile([128, TG], FP, tag="b_muS")
                nc.vector.tensor_reduce(muS[:, 0:nw7], s1b[:, 0:nw7, :],
                                        axis=AXX, op=ADD)
                mu = wp.tile([128, TG], BF, tag="b_mu")
                nc.scalar.activation(mu[:, 0:nw7], muS[:, 0:nw7], COPY,
                                     scale=1.0 / 128.0)
                nc.vector.tensor_tensor(
                    cen_all[:, g0:g0 + nw7, :], s1b[:, 0:nw7, :],
                    mu[:, 0:nw7].unsqueeze(2).broadcast_to([128, nw7, 128]),
                    SUB)
                sq = wp.tile([128, TG, 128], BF, tag="b_sq")
                nc.vector.tensor_tensor(sq[:, 0:nw7, :],
                                        cen_all[:, g0:g0 + nw7, :],
                                        cen_all[:, g0:g0 + nw7, :], MUL)
                nc.vector.tensor_reduce(var_all[:, g0:g0 + nw7],
                                        sq[:, 0:nw7, :], axis=AXX, op=ADD)
            std_all = wp.tile([128, NW], FP, tag="b_std")
            nc.scalar.activation(std_all[:], var_all[:], SQRT,
                                 bias=eps_s[:, 0:1], scale=1.0 / 128.0)
            rstd_all = wp.tile([128, NW], BF, tag="b_rstd")
            with nc.allow_low_precision(reason="bf16 rstd is plenty"):
                nc.vector.reciprocal(rstd_all[:], std_all[:])
            for g0 in range(0, NW, TG):
                nw7 = min(TG, NW - g0)
                nc.vector.tensor_tensor(
                    z_all[:, g0:g0 + nw7, :], cen_all[:, g0:g0 + nw7, :],
                    rstd_all[:, g0:g0 + nw7].unsqueeze(2)
                    .broadcast_to([128, nw7, 128]), MUL)

            # ---- stage C: xm = z @ w2mp + b2mp -------------------------
            for w in range(NW):
                pzt = pp2.tile([128, 128], BF, tag="pzt")
                nc.tensor.transpose(pzt[:], z_all[:, w, :], id_s[:])
                zt = wp.tile([128, 128], BF, tag="zt")
                nc.vector.tensor_copy(zt[:], pzt[:])
                pxm = pp.tile([128, 128], FP, tag="pxm")
                nc.tensor.matmul(pxm[:], zt[:], w2mp_s[:], start=True,
                                 stop=True)
                nc.vector.tensor_tensor(xm_acc[:, w, :], pxm[:], b2mpr_s[:],
                                        ADD)
            nc.sync.dma_start(xm_out, xm_acc[:])

    nc.compile()
    return nc


def _phase1_inputs(h_slots, inp):
    w2mp = inp["msg_ln_g"][:, None] * inp["msg_w2"]
    b2mp = inp["msg_ln_b"] @ inp["msg_w2"] + inp["msg_b2"]
    consts = {
        "linw": _bf(inp["lin_w"]),
        "linb_rep": _f32(np.broadcast_to(inp["lin_b"][None, :], (128, 128))),
        "linb_col": _f32(inp["lin_b"][:, None]),
        "w1m": _bf(inp["msg_w1"]),
        "b1m_rep": _f32(np.broadcast_to(inp["msg_b1"][None, :], (128, 128))),
        "w2mp": _bf(w2mp),
        "b2mp_rep": _f32(np.broadcast_to(b2mp[None, :], (128, 128))),
        "wr": _bf(inp["att_w1"][0:D]), "wc": _bf(inp["att_w1"][D:2 * D]),
        "b1a_rep": _f32(np.broadcast_to(inp["att_b1"][None, :], (128, 128))),
        "ident": _bf(np.eye(128)),
        "eps_col": _f32(np.full((128, 1), LN_EPS)),
    }
    in_maps = []
    for c in range(NCORES):
        in_maps.append({"ht": _bf(h_slots[c].T), **consts})
    return in_maps


def _run_phase1(h_slots, inp):
    if "p1" not in _cache:
        _cache["p1"] = _build_phase1()
    nc = _cache["p1"]
    res = run_bass_kernel_spmd(nc, _phase1_inputs(h_slots, inp),
                               core_ids=list(range(NCORES)), trace=TRACE)
    if TRACE:
        LAST_RESULTS["phase1_ns"] = res.exec_time_ns
        LAST_RESULTS["phase1_res"] = res
    return res.results


# ---------------------------------------------------------------------------
# Phase 2: edge stage + out stage
# ---------------------------------------------------------------------------

def _tile_table(sizes_lo, sizes_hi):
    """Per-tile (pass, window, first, last) in processing order."""
    tiles = []
    for p, sizes in ((0, sizes_lo), (1, sizes_hi)):
        for j, sz in enumerate(sizes):
            w = j if p == 0 else NW - 1 - j
            nt = sz // 128
            for t in range(nt):
                tiles.append((p, w, t == 0, t == nt - 1))
    return tiles


def _build_phase2(plan):
    sizes_lo, sizes_hi = plan["sizes_lo"], plan["sizes_hi"]
    EP_lo, EP_hi = sum(sizes_lo), sum(sizes_hi)
    EP = EP_lo + EP_hi
    tiles = _tile_table(sizes_lo, sizes_hi)
    NTILE = EP // 128
    NHI = N - SPLIT

    nc = bacc.Bacc("TRN2", target_bir_lowering=False, debug=False,
                   num_devices=NCORES)
    g = lambda n, s, d, k: nc.dram_tensor(n, s, d, kind=k).ap()

    tab_lo = g("tab_lo", [SPLIT, 256], BF, "ExternalInput")
    tab_hi = g("tab_hi", [NHI, 256], BF, "ExternalInput")
    aloc = g("aloc", [128, NLP], BF, "ExternalInput")
    xres = g("xres", [128, NLP], FP, "ExternalInput")
    idxt = g("idx", [128, EP // 16], I16, "ExternalInput")
    lrowr = g("lrow_row", [1, EP], BF, "ExternalInput")
    lrowc = g("lrowc", [128, EP // 128], BF, "ExternalInput")
    mbt = g("mb", [128, EP // 128], FP, "ExternalInput")
    ea4t = g("ea4", [4, EP // 4], BF, "ExternalInput")

    iof = g("iof", [128, 128], BF, "ExternalInput")
    iota_c = g("iota_col", [128, 1], FP, "ExternalInput")
    iota_cb = g("iota_colb", [128, 1], BF, "ExternalInput")
    ones1 = g("ones1", [1, 128], BF, "ExternalInput")
    ident = g("ident", [128, 128], BF, "ExternalInput")
    we4 = g("we4blk", [4, 512], BF, "ExternalInput")
    w2r = g("w2r", [128, 128], BF, "ExternalInput")
    w1o = g("w1o", [128, 128], BF, "ExternalInput")
    b1oc = g("b1o_col", [128, 1], FP, "ExternalInput")
    w2o = g("w2op", [128, 128], BF, "ExternalInput")
    b2o = g("b2op_row", [1, 128], BF, "ExternalInput")
    lngr = g("lng_rep", [128, 128], BF, "ExternalInput")
    lnbr = g("lnb_rep", [128, 128], BF, "ExternalInput")
    eps_c = g("eps_col", [128, 1], FP, "ExternalInput")

    out = g("out", [128, NLP], FP, "ExternalOutput")

    with tile.TileContext(nc) as tc:
        with tc.tile_pool(name="const", bufs=1) as cp, \
             tc.tile_pool(name="stream", bufs=1) as sp, \
             tc.tile_pool(name="acc", bufs=1) as accp, \
             tc.tile_pool(name="wk", bufs=2) as wk:

            # pools + first gathers FIRST so the Q7 descriptor mill starts
            # before the big const loads queue up
            chp = tc.alloc_tile_pool(name="chunk", bufs=4)
            idp = tc.alloc_tile_pool(name="idxp", bufs=4)
            selp = tc.alloc_tile_pool(name="selpool", bufs=8)

            chunks = []
            for p_, (p0, plen) in enumerate(((0, EP_lo), (EP_lo, EP_hi))):
                o_ = 0
                while o_ < plen:
                    gsz_ = min(CHUNK, plen - o_)
                    chunks.append((p0 + o_, gsz_, p_))
                    o_ += gsz_

            ck_tiles = {}

            def issue_gather(ci):
                off, gsz, p = chunks[ci]
                src = tab_lo if p == 0 else tab_hi
                idxc = idp.tile([128, CHUNK // 16], I16, tag="idxc")
                nc.sync.dma_start(idxc[:, 0:gsz // 16],
                                  idxt[:, off // 16:(off + gsz) // 16])
                t = chp.tile([128, CHUNK // 128, 256], BF, tag="cx")
                nc.gpsimd.dma_gather(
                    t[:, 0:gsz // 128, :], src,
                    idxc[:, 0:gsz // 16],
                    gsz, gsz, 256, single_packet=False)
                ck_tiles[ci] = (t, off)

            for ci_ in range(min(3, len(chunks))):
                issue_gather(ci_)

            def cload(ap, shape, dt, tag, pool=None):
                t = (pool or cp).tile(shape, dt, tag=tag)
                nc.sync.dma_start(t[:], ap)
                return t

            aloc_s = cload(aloc, [128, NLP], BF, "c_aloc")
            lrc_s = cload(lrowc, [128, EP // 128], BF, "c_lrc", sp)
            mb_s = cload(mbt, [128, EP // 128], FP, "c_mb", sp)
            iof_s = cload(iof, [128, 128], BF, "c_iof")
            ioc_s = cload(iota_c, [128, 1], FP, "c_ioc")
            iocb_s = cload(iota_cb, [128, 1], BF, "c_iocb")
            ones_s = cload(ones1, [1, 128], BF, "c_ones")
            id_s = cload(ident, [128, 128], BF, "c_id")
            we4_s = cload(we4, [4, 512], BF, "c_we4")
            w2r_s = cload(w2r, [128, 128], BF, "c_w2r")
            w1o_s = cload(w1o, [128, 128], BF, "c_w1o")
            b1o_s = cload(b1oc, [128, 1], FP, "c_b1o")
            w2o_s = cload(w2o, [128, 128], BF, "c_w2o")
            b2o_s = cload(b2o, [1, 128], BF, "c_b2o")
            lng_s = cload(lngr, [128, 128], BF, "c_lng")
            lnb_s = cload(lnbr, [128, 128], BF, "c_lnb")
            eps_s = cload(eps_c, [128, 1], FP, "c_eps")

            msgT = accp.tile([128, NW, 128], BF, tag="msgT")   # [d, w, slot]
            h1_all = accp.tile([128, NW, 128], BF, tag="h1all")

            ppp = tc.alloc_tile_pool(name="pprep", bufs=1, space="PSUM")
            php = tc.alloc_tile_pool(name="phid", bufs=2, space="PSUM")
            pmp = tc.alloc_tile_pool(name="pmsg", bufs=2, space="PSUM")
            pop = tc.alloc_tile_pool(name="pout", bufs=1, space="PSUM")

            msg_t = [None]
            closed_hi = []
            pending = [None]

            def part_a(ws):
                """out-net first linear + silu (sigmoid table) for windows ws
                (consecutive), into h1_all."""
                wmin = min(ws)
                nw = len(ws)
                p1 = pop.tile([128, 512], FP, tag="p1T")
                nc.tensor.matmul(p1[:, 0:nw * 128], w1o_s[:],
                                 msgT[:, wmin:wmin + nw, :],
                                 start=True, stop=True)
                sg = wk.tile([128, 512], BF, tag="a_sg")
                nc.scalar.activation(sg[:, 0:nw * 128], p1[:, 0:nw * 128],
                                     SIGM, bias=b1o_s[:, 0:1])
                ts_ = wk.tile([128, 512], BF, tag="a_ts")
                nc.scalar.activation(ts_[:, 0:nw * 128], p1[:, 0:nw * 128],
                                     IDEN, bias=b1o_s[:, 0:1])
                h1 = wk.tile([128, 512], BF, tag="a_h1")
                nc.vector.tensor_tensor(h1[:, 0:nw * 128], ts_[:, 0:nw * 128],
                                        sg[:, 0:nw * 128], MUL)
                for k, w in enumerate(sorted(ws)):
                    pt = pop.tile([128, 128], BF, tag="a_pt")
                    nc.tensor.transpose(pt[:], h1[:, k * 128:(k + 1) * 128],
                                        id_s[:])
                    nc.scalar.activation(h1_all[:, w, :], pt[:], COPY)

            def scatter_group(gstate):
                """Emit scatter matmuls + window closes for a finished chunk."""
                ci, sel_tiles = gstate
                ctile, coff = ck_tiles[ci]
                for mi, sel in sel_tiles:
                    for t in range(8):
                        gt = mi * 8 + t
                        p, w, first, last = tiles[gt]
                        loc = (gt * 128 - coff) // 128
                        if first:
                            msg_t[0] = pmp.tile([128, 128], FP, tag="msgw",
                                                name="msgw")
                        nc.tensor.matmul(msg_t[0][:],
                                         ctile[:, loc, 128:256],
                                         sel[:, t, :],
                                         start=first, stop=last)
                        if last:
                            if p == 0:
                                nc.scalar.activation(msgT[:, w, :],
                                                     msg_t[0][:], COPY)
                            else:
                                nc.vector.tensor_tensor(
                                    msgT[:, w, :], msg_t[0][:],
                                    msgT[:, w, :], ADD)
                                closed_hi.append(w)
                                if len(closed_hi) == 4 or w == 0:
                                    part_a(list(closed_hi))
                                    closed_hi.clear()

            for ci, (off, gsz, p) in enumerate(chunks):
                ctile, coff = ck_tiles[ci]
                # per-chunk row-side streams (full preload would blow SBUF)
                lrch = wk.tile([1, CHUNK], BF, tag="lrch")
                nc.sync.dma_start(lrch[0:1, 0:gsz], lrowr[0:1, off:off + gsz])
                each = wk.tile([4, CHUNK // 4], BF, tag="each")
                nc.sync.dma_start(each[:, 0:gsz // 4],
                                  ea4t[:, off // 4:(off + gsz) // 4])
                # scatter the PREVIOUS chunk first: its inputs are ready, so
                # the in-order PE does useful work while this chunk's gather
                # transfer completes
                if pending[0] is not None:
                    scatter_group(pending[0])
                    pending[0] = None
                gmacs = list(range(off // MACRO, (off + gsz) // MACRO))
                strip = wk.tile([128, 32], FP, tag="strip")
                for k, m in enumerate(gmacs):
                    moff = m * MACRO            # stream offset
                    # one-hot [s, e] for A-expand
                    prepb = wk.tile([128, 1024], BF, tag="prepb")
                    lmoff = moff - off
                    for h in range(2):
                        prep = ppp.tile([128, 512], FP, tag="prep")
                        nc.tensor.matmul(prep[:], ones_s[:],
                                         lrch[0:1, lmoff + h * 512:
                                              lmoff + (h + 1) * 512],
                                         start=True, stop=True)
                        nc.scalar.activation(prepb[:, h * 512:(h + 1) * 512],
                                             prep[:], COPY)
                    oht = wk.tile([128, 1024], BF, tag="oht")
                    nc.vector.tensor_tensor(
                        oht[:], prepb[:],
                        iocb_s[:, 0:1].broadcast_to([128, 1024]), EQ)
                    silu4 = wk.tile([128, 1024], BF, tag="silu4")
                    for h in range(2):
                        hid = php.tile([128, 512], FP, tag="hid")
                        # bank-wide opener first: start=True clears has_written
                        # for the WHOLE bank, so it must come before the
                        # per-region accumulating matmuls
                        m512 = lmoff // 512 + h
                        nc.tensor.matmul(hid[:],
                                         each[:, m512 * 128:(m512 + 1) * 128],
                                         we4_s[:], start=True, stop=False)
                        for t in range(4):
                            gt = m * 8 + h * 4 + t
                            w = tiles[gt][1]
                            nc.tensor.matmul(
                                hid[:, t * 128:(t + 1) * 128],
                                oht[:, (h * 4 + t) * 128:(h * 4 + t + 1) * 128],
                                aloc_s[:, w * 128:(w + 1) * 128],
                                start=False, stop=False)
                        loc0 = (moff - coff) // 128 + h * 4
                        nc.tensor.matmul(hid[:], id_s[:],
                                         ctile[:, loc0:loc0 + 4, 0:128],
                                         start=False, stop=True)
                        nc.scalar.activation(silu4[:, h * 512:(h + 1) * 512],
                                             hid[:], SILU)
                    sw = wk.tile([128, 8, 128], BF, tag="sw")
                    nc.vector.tensor_tensor(
                        sw[:],
                        silu4[:].rearrange("p (a b) -> p a b", b=128),
                        w2r_s[:].unsqueeze(1).broadcast_to([128, 8, 128]), MUL)
                    nc.vector.tensor_reduce(strip[:, k * 8:(k + 1) * 8],
                                            sw[:], axis=AXX, op=ADD)
                # strip -> att (one table flip per chunk)
                gt0 = gmacs[0] * 8
                ntl = len(gmacs) * 8
                stb = wk.tile([128, 32], FP, tag="stb")
                nc.vector.tensor_tensor(stb[:, 0:ntl], strip[:, 0:ntl],
                                        mb_s[:, gt0:gt0 + ntl], ADD)
                att = wk.tile([128, 32], BF, tag="att")
                nc.scalar.activation(att[:, 0:ntl], stb[:, 0:ntl], SIGM)
                # sel one-hots (att folded) for each macro
                sel_tiles = []
                for k, m in enumerate(gmacs):
                    se = wk.tile([128, 8, 128], BF, tag="selEQ")
                    nc.vector.tensor_tensor(
                        se[:],
                        iof_s[:].unsqueeze(1).broadcast_to([128, 8, 128]),
                        lrc_s[:, m * 8:(m + 1) * 8].unsqueeze(2)
                        .broadcast_to([128, 8, 128]), EQ)
                    sa = selp.tile([128, 8, 128], BF, tag="selA")
                    nc.vector.tensor_tensor(
                        sa[:], se[:],
                        att[:, k * 8:(k + 1) * 8].unsqueeze(2)
                        .broadcast_to([128, 8, 128]), MUL)
                    sel_tiles.append((m, sa))
                pending[0] = (ci, sel_tiles)
                # prefetch AFTER the previous chunk's readers are emitted so
                # buffer reuse is dependency-tracked
                if ci + 3 < len(chunks):
                    issue_gather(ci + 3)
            scatter_group(pending[0])

            for _p in (pop, pmp, php, ppp, selp, idp, chp):
                _p.release()

            # ---- tail: LayerNorms + second out-net linear + residual ------
            tp = tc.alloc_tile_pool(name="tailp", bufs=2, space="PSUM")
            tw = tc.alloc_tile_pool(name="tailw", bufs=2)
            TG = 7

            def ln_apply(src3, nw7, tagp):
                """LN over innermost 128 of src3 [128, nw7, 128] -> bf16 z."""
                muS = tw.tile([128, TG], FP, tag=f"{tagp}_muS")
                nc.vector.tensor_reduce(muS[:, 0:nw7], src3, axis=AXX, op=ADD)
                mu = tw.tile([128, TG], BF, tag=f"{tagp}_mu")
                nc.scalar.activation(mu[:, 0:nw7], muS[:, 0:nw7], COPY,
                                     scale=1.0 / 128.0)
                cen = tw.tile([128, TG, 128], BF, tag=f"{tagp}_cen")
                nc.vector.tensor_tensor(
                    cen[:, 0:nw7, :], src3,
                    mu[:, 0:nw7].unsqueeze(2).broadcast_to([128, nw7, 128]),
                    SUB)
                sq = tw.tile([128, TG, 128], BF, tag=f"{tagp}_sq")
                nc.vector.tensor_tensor(sq[:, 0:nw7, :], cen[:, 0:nw7, :],
                                        cen[:, 0:nw7, :], MUL)
                varS = tw.tile([128, TG], FP, tag=f"{tagp}_var")
                nc.vector.tensor_reduce(varS[:, 0:nw7], sq[:, 0:nw7, :],
                                        axis=AXX, op=ADD)
                std = tw.tile([128, TG], FP, tag=f"{tagp}_std")
                nc.scalar.activation(std[:, 0:nw7], varS[:, 0:nw7], SQRT,
                                     bias=eps_s[:, 0:1], scale=1.0 / 128.0)
                rstd = tw.tile([128, TG], BF, tag=f"{tagp}_rstd")
                with nc.allow_low_precision(reason="bf16 rstd is plenty"):
                    nc.vector.reciprocal(rstd[:, 0:nw7], std[:, 0:nw7])
                z = tw.tile([128, TG, 128], BF, tag=f"{tagp}_z")
                nc.vector.tensor_tensor(
                    z[:, 0:nw7, :], cen[:, 0:nw7, :],
                    rstd[:, 0:nw7].unsqueeze(2).broadcast_to([128, nw7, 128]),
                    MUL)
                return z

            for g0 in range(0, NW, TG):
                nw7 = min(TG, NW - g0)
                cs = slice(g0 * 128, (g0 + nw7) * 128)
                xr = tw.tile([128, TG * 128], FP, tag="t_xr")
                nc.sync.dma_start(xr[:, 0:nw7 * 128], xres[:, cs])

                z1 = ln_apply(h1_all[:, g0:g0 + nw7, :], nw7, "l1")
                r3 = tw.tile([128, TG, 128], BF, tag="t_r")
                for k in range(nw7):
                    pzt = tp.tile([128, 128], BF, tag="t_pzt")
                    nc.tensor.transpose(pzt[:], z1[:, k, :], id_s[:])
                    zt = tw.tile([128, 128], BF, tag="t_zt")
                    nc.scalar.activation(zt[:], pzt[:], COPY)
                    po2 = tp.tile([128, 128], FP, tag="t_po2")
                    nc.tensor.matmul(po2[:], zt[:], w2o_s[:], start=True,
                                     stop=False)
                    nc.tensor.matmul(po2[:], ones_s[:], b2o_s[:], start=False,
                                     stop=True)
                    nc.vector.tensor_tensor(
                        r3[:, k, :], po2[:],
                        xr[:, k * 128:(k + 1) * 128], ADD)
                z2 = ln_apply(r3[:, 0:nw7, :], nw7, "l2")
                zg = tw.tile([128, TG, 128], BF, tag="t_zg")
                nc.vector.tensor_tensor(
                    zg[:, 0:nw7, :], z2[:, 0:nw7, :],
                    lng_s[:].unsqueeze(1).broadcast_to([128, nw7, 128]), MUL)
                og = tw.tile([128, TG, 128], FP, tag="t_og")
                nc.vector.tensor_tensor(
                    og[:, 0:nw7, :], zg[:, 0:nw7, :],
                    lnb_s[:].unsqueeze(1).broadcast_to([128, nw7, 128]), ADD)
                nc.sync.dma_start(out[:, cs], og[:, 0:nw7, :])
            tw.release()
            tp.release()

    nc.compile()
    return nc


def _phase2_inputs(plan, streams, p1_results, inp, slot_node):
    att_w1, att_w2 = inp["att_w1"], inp["att_w2"]

    def unwrap(a, c):
        a = np.asarray(a, dtype=F32)
        return a.reshape(128, NW, 128).transpose(1, 0, 2).reshape(NLP, 128)

    # merged [B ; xm] table in original node order
    tab = np.zeros((N, 256), F32)
    for c in range(NCORES):
        bl = unwrap(p1_results[c]["b_out"], c)
        xl = unwrap(p1_results[c]["xm_out"], c)
        sn = slot_node[c]
        m = sn >= 0
        tab[sn[m], 0:128] = bl[m]
        tab[sn[m], 128:256] = xl[m]
    tab = _bf(tab)

    w2op = inp["out_ln_g"][:, None] * inp["out_w2"]
    b2op = inp["out_ln_b"] @ inp["out_w2"] + inp["out_b2"]
    we = att_w1[2 * D:2 * D + 1].ravel()       # [128]
    we4 = np.zeros((4, 512), F32)
    for k in range(4):
        we4[k, k * 128:(k + 1) * 128] = we

    consts = {
        "tab_lo": tab[0:SPLIT], "tab_hi": tab[SPLIT:N],
        "iof": _bf(np.broadcast_to(np.arange(128)[None, :], (128, 128))),
        "iota_col": _f32(np.arange(128)[:, None]),
        "iota_colb": _bf(np.arange(128)[:, None]),
        "ones1": _bf(np.ones((1, 128))), "ident": _bf(np.eye(128)),
        "we4blk": _bf(we4),
        "w2r": _bf(np.broadcast_to(att_w2[:, 0][None, :], (128, 128))),
        "w1o": _bf(inp["out_w1"]), "b1o_col": _f32(inp["out_b1"][:, None]),
        "w2op": _bf(w2op), "b2op_row": _bf(b2op[None, :]),
        "lng_rep": _bf(np.broadcast_to(inp["ln_g"][None, :], (128, 128))),
        "lnb_rep": _bf(np.broadcast_to(inp["ln_b"][None, :], (128, 128))),
        "eps_col": _f32(np.full((128, 1), LN_EPS)),
    }
    in_maps = []
    for c in range(NCORES):
        in_maps.append({
            "aloc": np.ascontiguousarray(np.asarray(p1_results[c]["a_out"])),
            "xres": np.ascontiguousarray(np.asarray(p1_results[c]["x_out"])),
            "idx": streams["idx"][c],
            "lrow_row": streams["lrow_row"][c],
            "lrowc": streams["lrowc"][c],
            "mb": streams["mb"][c],
            "ea4": streams["ea4"][c],
            **consts,
        })
    return in_maps


def kernel(**inputs):
    inp = {k: np.asarray(v) for k, v in inputs.items()}
    b2 = float(np.asarray(inp["att_b2"]).ravel()[0])
    plan, streams, slot_node = _prep_edges(
        inp["row"], inp["col"], inp["edge_attr"], inp["edge_mask"], b2)

    # phase-1 input: h gathered into (core, slot) layout
    h = _f32(inp["h"])
    h_slots = np.zeros((NCORES, NLP, D), F32)
    for c in range(NCORES):
        sn = slot_node[c]
        m = sn >= 0
        h_slots[c][m] = h[sn[m]]

    p1 = _run_phase1(h_slots, inp)

    key = (plan["sizes_lo"], plan["sizes_hi"])
    if _cache.get("p2_key") != key:
        _cache["p2"] = _build_phase2(plan)
        _cache["p2_key"] = key
    nc2 = _cache["p2"]
    in_maps = _phase2_inputs(plan, streams, p1, inp, slot_node)
    res = run_bass_kernel_spmd(nc2, in_maps, core_ids=list(range(NCORES)),
                               trace=TRACE)
    LAST_RESULTS["phase2_results"] = res.results
    if TRACE:
        LAST_RESULTS["phase2_ns"] = res.exec_time_ns
        LAST_RESULTS["phase2_res"] = res

    out = np.zeros((N, D), F32)
    for c in range(NCORES):
        o = np.asarray(res.results[c]["out"], F32)
        o = o.reshape(128, NW, 128).transpose(1, 0, 2).reshape(NLP, 128)
        sn = slot_node[c]
        m = sn >= 0
        out[sn[m]] = o[m]
    return out.astype(F32)


# revision 58
# speedup vs baseline: 2.8974x; 1.0177x over previous
"""GCLayer (GNN message passing) on 8 Trainium2 NeuronCores — v2.

Strategy (see spec sharding_hint): edges partitioned by destination row;
node features replicated host-side between the two NEFFs.

Key points vs v1:
- ONE merged dma_gather per edge chunk fetching 512B rows [B[col] ; xm[col]]
  (B = x@wc precomputed in phase 1).  Q7 descriptor generation is the
  hard wall (~8 ns/desc), so descriptors are halved vs two 256B gathers.
- Node->(core,window,slot) assignment is load-balanced host-side so the
  static per-window tile counts carry ~2% padding instead of ~19%.
- Edge math is batched into 1024-edge macro-tiles: one DVE instruction per
  one-hot build / silu-product / logit-reduce; scatter via T-layout one-hot
  matmuls; silu runs directly on ACT with per-chunk table phases (no
  per-tile Silu<->Sigmoid table thrash).
- hi-pass windows run in reverse so per-window out-net work interleaves
  with the tail of the gather stream; LayerNorms run stage-major with
  exactly one Sqrt table load.
"""

import sys

sys.path.insert(0, "/opt/trn_rl_repo")

import numpy as np
import ml_dtypes

from concourse import bacc, mybir, tile
from concourse.bass_utils import run_bass_kernel_spmd

BF16 = ml_dtypes.bfloat16
F32 = np.float32

NCORES = 8
N = 50000
E = 800000
D = 128
NW = 49                   # windows (128 slots) per core
NLP = NW * 128            # 6272 slots per core
NBINS = NCORES * NW       # 392
SPLIT = 32768             # int16 gather index limit
CHUNK = 4096              # edges per dma_gather call == strip group
MACRO = 1024              # edges per DVE macro-tile
LN_EPS = 1e-5
MASK_BIAS = -30000.0

FP = mybir.dt.float32
BF = mybir.dt.bfloat16
I16 = mybir.dt.int16

SILU = mybir.ActivationFunctionType.Silu
IDEN = mybir.ActivationFunctionType.Identity
SIGM = mybir.ActivationFunctionType.Sigmoid
SQRT = mybir.ActivationFunctionType.Sqrt
COPY = mybir.ActivationFunctionType.Copy
EQ = mybir.AluOpType.is_equal
MUL = mybir.AluOpType.mult
ADD = mybir.AluOpType.add
SUB = mybir.AluOpType.subtract
AXX = mybir.AxisListType.X

TRACE = False
LAST_RESULTS = {}

_cache = {}


def _bf(a):
    return np.ascontiguousarray(np.asarray(a, dtype=F32).astype(BF16))


def _f32(a):
    return np.ascontiguousarray(np.asarray(a, dtype=F32))


def _ceil(a, m):
    return -(-int(a) // m) * m


# ---------------------------------------------------------------------------
# Host-side preprocessing
# ---------------------------------------------------------------------------

def _balance_nodes(row, col):
    """Assign nodes to (core, window, slot) balancing per-bin lo/hi edge
    counts.  Returns slot_node [NCORES, NLP] (node id or -1) and
    (c, w, s) arrays per node."""
    hi = col >= SPLIT
    d_lo = np.bincount(row[~hi], minlength=N).astype(np.int64)
    d_hi = np.bincount(row[hi], minlength=N).astype(np.int64)
    order = np.argsort(-(d_lo + d_hi), kind="stable")

    lo_sum = np.zeros(NBINS)
    hi_sum = np.zeros(NBINS)
    a_lo = max(d_lo.sum() / NBINS, 1.0)
    a_hi = max(d_hi.sum() / NBINS, 1.0)
    slot_node = np.full((NBINS, 128), -1, np.int64)
    node_bin = np.empty(N, np.int64)
    node_slot = np.empty(N, np.int64)
    for r in range(128):
        chunk = order[r * NBINS:(r + 1) * NBINS]
        if chunk.size == 0:
            break
        load = lo_sum / a_lo + hi_sum / a_hi
        bins = np.argsort(load, kind="stable")[:chunk.size]
        # heaviest nodes -> least-loaded bins (chunk already degree-sorted)
        slot_node[bins, r] = chunk
        node_bin[chunk] = bins
        node_slot[chunk] = r
        lo_sum[bins] += d_lo[chunk]
        hi_sum[bins] += d_hi[chunk]
    c_of = node_bin // NW
    w_of = node_bin % NW
    return slot_node.reshape(NCORES, NLP), c_of, w_of, node_slot


def _prep_edges(row, col, ea, em, b2):
    row = np.asarray(row).astype(np.int64).ravel()
    col = np.asarray(col).astype(np.int64).ravel()
    ea = np.asarray(ea, dtype=F32).ravel()
    em = np.asarray(em, dtype=F32).ravel()

    slot_node, c_of_n, w_of_n, s_of_n = _balance_nodes(row, col)
    c_of = c_of_n[row]
    w_of = w_of_n[row]
    s_in_w = s_of_n[row]
    hi = (col >= SPLIT).astype(np.int64)

    # processing order: lo pass windows ascending, hi pass windows descending
    worder = np.where(hi == 0, w_of, NW - 1 - w_of)
    key = (c_of * 2 + hi) * NW + worder
    order = np.argsort(key * (1 << 17) + col, kind="stable")
    skey = key[order]
    cnt = np.bincount(key, minlength=NCORES * 2 * NW).reshape(NCORES, 2, NW)

    # padded per-processing-position sizes (shared across cores)
    sizes_lo = np.array([_ceil(cnt[:, 0, j].max(), 128) for j in range(NW)])
    sizes_hi = np.array([_ceil(cnt[:, 1, j].max(), 128) for j in range(NW)])
    assert (sizes_lo > 0).all() and (sizes_hi > 0).all()
    sizes_lo[-1] += _ceil(sizes_lo.sum(), MACRO) - sizes_lo.sum()
    sizes_hi[-1] += _ceil(sizes_hi.sum(), MACRO) - sizes_hi.sum()
    EP_lo = int(sizes_lo.sum())
    EP_hi = int(sizes_hi.sum())
    EP = EP_lo + EP_hi

    sizes = np.concatenate([sizes_lo, sizes_hi])          # (2*NW,) proc order
    base = np.concatenate([[0], np.cumsum(sizes)[:-1]])

    bstart = np.concatenate([[0], np.cumsum(cnt.reshape(NCORES, -1)
                                            .reshape(-1))[:-1]])
    rank = np.arange(row.size) - bstart[skey]
    pw = skey % (2 * NW)
    dest = base[pw] + rank
    cc = skey // (2 * NW)

    g_lrow = np.zeros((NCORES, EP), F32)
    g_idx = np.zeros((NCORES, EP), np.int16)
    g_ea = np.zeros((NCORES, EP), F32)
    g_mb = np.full((NCORES, EP), b2 + MASK_BIAS, F32)

    e_ids = order
    g_lrow[cc, dest] = s_in_w[e_ids].astype(F32)
    g_idx[cc, dest] = (col[e_ids] - hi[e_ids] * SPLIT).astype(np.int16)
    g_ea[cc, dest] = ea[e_ids]
    g_mb[cc, dest] = b2 + (em[e_ids] - 1.0) * (-MASK_BIAS) * (-1.0)

    idx16 = g_idx.reshape(NCORES, EP // 16, 16).transpose(0, 2, 1)
    streams = dict(
        idx=np.ascontiguousarray(np.tile(idx16, (1, 8, 1))),
        lrow_row=_bf(g_lrow.reshape(NCORES, 1, EP)),
        lrowc=_bf(g_lrow.reshape(NCORES, EP // 128, 128).transpose(0, 2, 1)),
        mb=_f32(g_mb.reshape(NCORES, EP // 128, 128).transpose(0, 2, 1)),
        ea4=_bf(g_ea.reshape(NCORES, EP // 512, 4, 128)
                .transpose(0, 2, 1, 3).reshape(NCORES, 4, EP // 4)),
    )
    plan = dict(sizes_lo=tuple(int(x) for x in sizes_lo),
                sizes_hi=tuple(int(x) for x in sizes_hi))
    return plan, streams, slot_node


# ---------------------------------------------------------------------------
# Phase 1: node stage (same structure as v1 + B = x@wc output)
# ---------------------------------------------------------------------------

def _build_phase1():
    nc = bacc.Bacc("TRN2", target_bir_lowering=False, debug=False,
                   num_devices=NCORES)
    g = lambda n, s, d, k: nc.dram_tensor(n, s, d, kind=k).ap()

    ht = g("ht", [128, NLP], BF, "ExternalInput")
    linw = g("linw", [128, 128], BF, "ExternalInput")
    linb_r = g("linb_rep", [128, 128], FP, "ExternalInput")
    linb_c = g("linb_col", [128, 1], FP, "ExternalInput")
    w1m = g("w1m", [128, 128], BF, "ExternalInput")
    b1m_r = g("b1m_rep", [128, 128], FP, "ExternalInput")
    w2mp = g("w2mp", [128, 128], BF, "ExternalInput")
    b2mp_r = g("b2mp_rep", [128, 128], FP, "ExternalInput")
    wr = g("wr", [128, 128], BF, "ExternalInput")
    wc = g("wc", [128, 128], BF, "ExternalInput")
    b1a_r = g("b1a_rep", [128, 128], FP, "ExternalInput")
    ident = g("ident", [128, 128], BF, "ExternalInput")
    eps_c = g("eps_col", [128, 1], FP, "ExternalInput")

    x_out = g("x_out", [128, NLP], FP, "ExternalOutput")
    xm_out = g("xm_out", [128, NLP], BF, "ExternalOutput")
    a_out = g("a_out", [128, NLP], BF, "ExternalOutput")
    b_out = g("b_out", [128, NLP], BF, "ExternalOutput")

    TG = 7

    with tile.TileContext(nc) as tc:
        with tc.tile_pool(name="const", bufs=1) as cp, \
             tc.tile_pool(name="acc", bufs=1) as ac, \
             tc.tile_pool(name="work", bufs=3) as wp, \
             tc.tile_pool(name="psum", bufs=1, space="PSUM") as pp, \
             tc.tile_pool(name="psum2", bufs=2, space="PSUM") as pp2:

            def cload(ap, shape, dt, tag):
                t = cp.tile(shape, dt, tag=tag)
                nc.sync.dma_start(t[:], ap)
                return t

            ht_s = cload(ht, [128, NLP], BF, "c_ht")
            linw_s = cload(linw, [128, 128], BF, "c_linw")
            linbr_s = cload(linb_r, [128, 128], FP, "c_linbr")
            linbc_s = cload(linb_c, [128, 1], FP, "c_linbc")
            w1m_s = cload(w1m, [128, 128], BF, "c_w1m")
            b1mr_s = cload(b1m_r, [128, 128], FP, "c_b1mr")
            w2mp_s = cload(w2mp, [128, 128], BF, "c_w2mp")
            b2mpr_s = cload(b2mp_r, [128, 128], FP, "c_b2mpr")
            wr_s = cload(wr, [128, 128], BF, "c_wr")
            wc_s = cload(wc, [128, 128], BF, "c_wc")
            b1ar_s = cload(b1a_r, [128, 128], FP, "c_b1ar")
            id_s = cload(ident, [128, 128], BF, "c_id")
            eps_s = cload(eps_c, [128, 1], FP, "c_eps")

            s1_all = ac.tile([128, NW, 128], FP, tag="s1all")
            xt_all = ac.tile([128, NW, 128], BF, tag="xtall")
            z_all = ac.tile([128, NW, 128], BF, tag="zall")
            x_acc = ac.tile([128, NW, 128], FP, tag="xacc")
            a_acc = ac.tile([128, NW, 128], BF, tag="aacc")
            b_acc = ac.tile([128, NW, 128], BF, tag="bacc")
            xm_acc = ac.tile([128, NW, 128], BF, tag="xmacc")

            # ---- stage A: per-window matmuls ---------------------------
            for w in range(NW):
                blk = slice(w * 128, (w + 1) * 128)
                htb = ht_s[:, blk]

                px = pp.tile([128, 128], FP, tag="px")
                nc.tensor.matmul(px[:], htb, linw_s[:], start=True, stop=True)
                nc.vector.tensor_tensor(x_acc[:, w, :], px[:], linbr_s[:], ADD)

                pxt = pp.tile([128, 128], FP, tag="pxt")
                nc.tensor.matmul(pxt[:], linw_s[:], htb, start=True, stop=True)
                nc.scalar.activation(xt_all[:, w, :], pxt[:], IDEN,
                                     bias=linbc_s[:, 0:1])

                pa = pp.tile([128, 128], FP, tag="pa")
                nc.tensor.matmul(pa[:], xt_all[:, w, :], wr_s[:],
                                 start=True, stop=True)
                nc.vector.tensor_tensor(a_acc[:, w, :], pa[:], b1ar_s[:], ADD)

                pb = pp.tile([128, 128], FP, tag="pb")
                nc.tensor.matmul(pb[:], xt_all[:, w, :], wc_s[:],
                                 start=True, stop=True)
                nc.vector.tensor_copy(b_acc[:, w, :], pb[:])

                ps = pp.tile([128, 128], FP, tag="ps")
                nc.tensor.matmul(ps[:], xt_all[:, w, :], w1m_s[:],
                                 start=True, stop=True)
                nc.vector.tensor_tensor(s1_all[:, w, :], ps[:], b1mr_s[:], ADD)
            nc.sync.dma_start(x_out, x_acc[:])
            nc.sync.dma_start(a_out, a_acc[:])
            nc.sync.dma_start(b_out, b_acc[:])

            # ---- stage B: batched silu + LN (2 table loads total) ------
            var_all = ac.tile([128, NW], FP, tag="varall")
            cen_all = ac.tile([128, NW, 128], BF, tag="cenall")
            for g0 in range(0, NW, TG):
                nw7 = min(TG, NW - g0)
                src = s1_all[:, g0:g0 + nw7, :]
                sg = wp.tile([128, TG, 128], BF, tag="b_sg")
                nc.scalar.activation(sg[:, 0:nw7, :], src, SIGM)
                s1b = wp.tile([128, TG, 128], BF, tag="b_s1b")
                nc.vector.tensor_tensor(s1b[:, 0:nw7, :], src,
                                        sg[:, 0:nw7, :], MUL)
                muS = wp.tile([128, TG], FP, tag="b_muS")
                nc.vector.tensor_reduce(muS[:, 0:nw7], s1b[:, 0:nw7, :],
                                        axis=AXX, op=ADD)
                mu = wp.tile([128, TG], BF, tag="b_mu")
                nc.scalar.activation(mu[:, 0:nw7], muS[:, 0:nw7], COPY,
                                     scale=1.0 / 128.0)
                nc.vector.tensor_tensor(
                    cen_all[:, g0:g0 + nw7, :], s1b[:, 0:nw7, :],
                    mu[:, 0:nw7].unsqueeze(2).broadcast_to([128, nw7, 128]),
                    SUB)
                sq = wp.tile([128, TG, 128], BF, tag="b_sq")
                nc.vector.tensor_tensor(sq[:, 0:nw7, :],
                                        cen_all[:, g0:g0 + nw7, :],
                                        cen_all[:, g0:g0 + nw7, :], MUL)
                nc.vector.tensor_reduce(var_all[:, g0:g0 + nw7],
                                        sq[:, 0:nw7, :], axis=AXX, op=ADD)
            std_all = wp.tile([128, NW], FP, tag="b_std")
            nc.scalar.activation(std_all[:], var_all[:], SQRT,
                                 bias=eps_s[:, 0:1], scale=1.0 / 128.0)
            rstd_all = wp.tile([128, NW], BF, tag="b_rstd")
            with nc.allow_low_precision(reason="bf16 rstd is plenty"):
                nc.vector.reciprocal(rstd_all[:], std_all[:])
            for g0 in range(0, NW, TG):
                nw7 = min(TG, NW - g0)
                nc.vector.tensor_tensor(
                    z_all[:, g0:g0 + nw7, :], cen_all[:, g0:g0 + nw7, :],
                    rstd_all[:, g0:g0 + nw7].unsqueeze(2)
                    .broadcast_to([128, nw7, 128]), MUL)

            # ---- stage C: xm = z @ w2mp + b2mp -------------------------
            for w in range(NW):
                pzt = pp2.tile([128, 128], BF, tag="pzt")
                nc.tensor.transpose(pzt[:], z_all[:, w, :], id_s[:])
                zt = wp.tile([128, 128], BF, tag="zt")
                nc.vector.tensor_copy(zt[:], pzt[:])
                pxm = pp.tile([128, 128], FP, tag="pxm")
                nc.tensor.matmul(pxm[:], zt[:], w2mp_s[:], start=True,
                                 stop=True)
                nc.vector.tensor_tensor(xm_acc[:, w, :], pxm[:], b2mpr_s[:],
                                        ADD)
            nc.sync.dma_start(xm_out, xm_acc[:])

    nc.compile()
    return nc


def _phase1_inputs(h_slots, inp):
    w2mp = inp["msg_ln_g"][:, None] * inp["msg_w2"]
    b2mp = inp["msg_ln_b"] @ inp["msg_w2"] + inp["msg_b2"]
    consts = {
        "linw": _bf(inp["lin_w"]),
        "linb_rep": _f32(np.broadcast_to(inp["lin_b"][None, :], (128, 128))),
        "linb_col": _f32(inp["lin_b"][:, None]),
        "w1m": _bf(inp["msg_w1"]),
        "b1m_rep": _f32(np.broadcast_to(inp["msg_b1"][None, :], (128, 128))),
        "w2mp": _bf(w2mp),
        "b2mp_rep": _f32(np.broadcast_to(b2mp[None, :], (128, 128))),
        "wr": _bf(inp["att_w1"][0:D]), "wc": _bf(inp["att_w1"][D:2 * D]),
        "b1a_rep": _f32(np.broadcast_to(inp["att_b1"][None, :], (128, 128))),
        "ident": _bf(np.eye(128)),
        "eps_col": _f32(np.full((128, 1), LN_EPS)),
    }
    in_maps = []
    for c in range(NCORES):
        in_maps.append({"ht": _bf(h_slots[c].T), **consts})
    return in_maps


def _run_phase1(h_slots, inp):
    if "p1" not in _cache:
        _cache["p1"] = _build_phase1()
    nc = _cache["p1"]
    res = run_bass_kernel_spmd(nc, _phase1_inputs(h_slots, inp),
                               core_ids=list(range(NCORES)), trace=TRACE)
    if TRACE:
        LAST_RESULTS["phase1_ns"] = res.exec_time_ns
        LAST_RESULTS["phase1_res"] = res
    return res.results


# ---------------------------------------------------------------------------
# Phase 2: edge stage + out stage
# ---------------------------------------------------------------------------

def _tile_table(sizes_lo, sizes_hi):
    """Per-tile (pass, window, first, last) in processing order."""
    tiles = []
    for p, sizes in ((0, sizes_lo), (1, sizes_hi)):
        for j, sz in enumerate(sizes):
            w = j if p == 0 else NW - 1 - j
            nt = sz // 128
            for t in range(nt):
                tiles.append((p, w, t == 0, t == nt - 1))
    return tiles


def _build_phase2(plan):
    sizes_lo, sizes_hi = plan["sizes_lo"], plan["sizes_hi"]
    EP_lo, EP_hi = sum(sizes_lo), sum(sizes_hi)
    EP = EP_lo + EP_hi
    tiles = _tile_table(sizes_lo, sizes_hi)
    NTILE = EP // 128
    NHI = N - SPLIT

    nc = bacc.Bacc("TRN2", target_bir_lowering=False, debug=False,
                   num_devices=NCORES)
    g = lambda n, s, d, k: nc.dram_tensor(n, s, d, kind=k).ap()

    tab_lo = g("tab_lo", [SPLIT, 256], BF, "ExternalInput")
    tab_hi = g("tab_hi", [NHI, 256], BF, "ExternalInput")
    aloc = g("aloc", [128, NLP], BF, "ExternalInput")
    xres = g("xres", [128, NLP], FP, "ExternalInput")
    idxt = g("idx", [128, EP // 16], I16, "ExternalInput")
    lrowr = g("lrow_row", [1, EP], BF, "ExternalInput")
    lrowc = g("lrowc", [128, EP // 128], BF, "ExternalInput")
    mbt = g("mb", [128, EP // 128], FP, "ExternalInput")
    ea4t = g("ea4", [4, EP // 4], BF, "ExternalInput")

    iof = g("iof", [128, 128], BF, "ExternalInput")
    iota_c = g("iota_col", [128, 1], FP, "ExternalInput")
    iota_cb = g("iota_colb", [128, 1], BF, "ExternalInput")
    ones1 = g("ones1", [1, 128], BF, "ExternalInput")
    ident = g("ident", [128, 128], BF, "ExternalInput")
    we4 = g("we4blk", [4, 512], BF, "ExternalInput")
    w2r = g("w2r", [128, 128], BF, "ExternalInput")
    w1o = g("w1o", [128, 128], BF, "ExternalInput")
    b1oc = g("b1o_col", [128, 1], FP, "ExternalInput")
    w2o = g("w2op", [128, 128], BF, "ExternalInput")
    b2o = g("b2op_row", [1, 128], BF, "ExternalInput")
    lngr = g("lng_rep", [128, 128], BF, "ExternalInput")
    lnbr = g("lnb_rep", [128, 128], BF, "ExternalInput")
    eps_c = g("eps_col", [128, 1], FP, "ExternalInput")

    out = g("out", [128, NLP], FP, "ExternalOutput")

    with tile.TileContext(nc) as tc:
        with tc.tile_pool(name="const", bufs=1) as cp, \
             tc.tile_pool(name="stream", bufs=1) as sp, \
             tc.tile_pool(name="acc", bufs=1) as accp, \
             tc.tile_pool(name="wk", bufs=2) as wk:

            # pools + first gathers FIRST so the Q7 descriptor mill starts
            # before the big const loads queue up
            chp = tc.alloc_tile_pool(name="chunk", bufs=3)
            idp = tc.alloc_tile_pool(name="idxp", bufs=3)
            selp = tc.alloc_tile_pool(name="selpool", bufs=8)

            chunks = []
            for p_, (p0, plen) in enumerate(((0, EP_lo), (EP_lo, EP_hi))):
                o_ = 0
                while o_ < plen:
                    gsz_ = min(CHUNK, plen - o_)
                    chunks.append((p0 + o_, gsz_, p_))
                    o_ += gsz_

            ck_tiles = {}

            def issue_gather(ci):
                off, gsz, p = chunks[ci]
                src = tab_lo if p == 0 else tab_hi
                idxc = idp.tile([128, CHUNK // 16], I16, tag="idxc")
                nc.sync.dma_start(idxc[:, 0:gsz // 16],
                                  idxt[:, off // 16:(off + gsz) // 16])
                t = chp.tile([128, CHUNK // 128, 256], BF, tag="cx")
                nc.gpsimd.dma_gather(
                    t[:, 0:gsz // 128, :], src,
                    idxc[:, 0:gsz // 16],
                    gsz, gsz, 256, single_packet=False)
                ck_tiles[ci] = (t, off)

            for ci_ in range(min(2, len(chunks))):
                issue_gather(ci_)

            def cload(ap, shape, dt, tag, pool=None):
                t = (pool or cp).tile(shape, dt, tag=tag)
                nc.sync.dma_start(t[:], ap)
                return t

            aloc_s = cload(aloc, [128, NLP], BF, "c_aloc")
            lrc_s = cload(lrowc, [128, EP // 128], BF, "c_lrc", sp)
            mb_s = cload(mbt, [128, EP // 128], FP, "c_mb", sp)
            iof_s = cload(iof, [128, 128], BF, "c_iof")
            ioc_s = cload(iota_c, [128, 1], FP, "c_ioc")
            iocb_s = cload(iota_cb, [128, 1], BF, "c_iocb")
            ones_s = cload(ones1, [1, 128], BF, "c_ones")
            id_s = cload(ident, [128, 128], BF, "c_id")
            we4_s = cload(we4, [4, 512], BF, "c_we4")
            w2r_s = cload(w2r, [128, 128], BF, "c_w2r")
            w1o_s = cload(w1o, [128, 128], BF, "c_w1o")
            b1o_s = cload(b1oc, [128, 1], FP, "c_b1o")
            w2o_s = cload(w2o, [128, 128], BF, "c_w2o")
            b2o_s = cload(b2o, [1, 128], BF, "c_b2o")
            lng_s = cload(lngr, [128, 128], BF, "c_lng")
            lnb_s = cload(lnbr, [128, 128], BF, "c_lnb")
            eps_s = cload(eps_c, [128, 1], FP, "c_eps")

            msgT = accp.tile([128, NW, 128], BF, tag="msgT")   # [d, w, slot]
            h1_all = accp.tile([128, NW, 128], BF, tag="h1all")

            ppp = tc.alloc_tile_pool(name="pprep", bufs=1, space="PSUM")
            php = tc.alloc_tile_pool(name="phid", bufs=1, space="PSUM")
            pmp = tc.alloc_tile_pool(name="pmsg", bufs=2, space="PSUM")
            pop = tc.alloc_tile_pool(name="pout", bufs=1, space="PSUM")
            tp = tc.alloc_tile_pool(name="tailp", bufs=1, space="PSUM")
            tw = tc.alloc_tile_pool(name="tailw", bufs=2)
            TG = 7

            def ln_apply(src3, nw7, tagp):
                """LN over innermost 128 of src3 [128, nw7, 128] -> bf16 z."""
                muS = tw.tile([128, TG], FP, tag=f"{tagp}_muS")
                nc.vector.tensor_reduce(muS[:, 0:nw7], src3, axis=AXX, op=ADD)
                mu = tw.tile([128, TG], BF, tag=f"{tagp}_mu")
                nc.scalar.activation(mu[:, 0:nw7], muS[:, 0:nw7], COPY,
                                     scale=1.0 / 128.0)
                cen = tw.tile([128, TG, 128], BF, tag=f"{tagp}_cen")
                nc.vector.tensor_tensor(
                    cen[:, 0:nw7, :], src3,
                    mu[:, 0:nw7].unsqueeze(2).broadcast_to([128, nw7, 128]),
                    SUB)
                sq = tw.tile([128, TG, 128], BF, tag=f"{tagp}_sq")
                nc.vector.tensor_tensor(sq[:, 0:nw7, :], cen[:, 0:nw7, :],
                                        cen[:, 0:nw7, :], MUL)
                varS = tw.tile([128, TG], FP, tag=f"{tagp}_var")
                nc.vector.tensor_reduce(varS[:, 0:nw7], sq[:, 0:nw7, :],
                                        axis=AXX, op=ADD)
                std = tw.tile([128, TG], FP, tag=f"{tagp}_std")
                nc.scalar.activation(std[:, 0:nw7], varS[:, 0:nw7], SQRT,
                                     bias=eps_s[:, 0:1], scale=1.0 / 128.0)
                rstd = tw.tile([128, TG], BF, tag=f"{tagp}_rstd")
                with nc.allow_low_precision(reason="bf16 rstd is plenty"):
                    nc.vector.reciprocal(rstd[:, 0:nw7], std[:, 0:nw7])
                z = tw.tile([128, TG, 128], BF, tag=f"{tagp}_z")
                nc.vector.tensor_tensor(
                    z[:, 0:nw7, :], cen[:, 0:nw7, :],
                    rstd[:, 0:nw7].unsqueeze(2).broadcast_to([128, nw7, 128]),
                    MUL)
                return z

            def tail_group(g0):
                """LN1 + out-net 2nd linear + residual + LN2 + store for
                windows [g0, g0+TG)."""
                nw7 = min(TG, NW - g0)
                cs = slice(g0 * 128, (g0 + nw7) * 128)
                xr = tw.tile([128, TG * 128], FP, tag="t_xr")
                nc.sync.dma_start(xr[:, 0:nw7 * 128], xres[:, cs])
                z1 = ln_apply(h1_all[:, g0:g0 + nw7, :], nw7, "l1")
                r3 = tw.tile([128, TG, 128], BF, tag="t_r")
                for k in range(nw7):
                    pzt = tp.tile([128, 128], BF, tag="t_pzt")
                    nc.tensor.transpose(pzt[:], z1[:, k, :], id_s[:])
                    zt = tw.tile([128, 128], BF, tag="t_zt")
                    nc.scalar.activation(zt[:], pzt[:], COPY)
                    po2 = tp.tile([128, 128], FP, tag="t_po2")
                    nc.tensor.matmul(po2[:], zt[:], w2o_s[:], start=True,
                                     stop=False)
                    nc.tensor.matmul(po2[:], ones_s[:], b2o_s[:], start=False,
                                     stop=True)
                    nc.vector.tensor_tensor(
                        r3[:, k, :], po2[:],
                        xr[:, k * 128:(k + 1) * 128], ADD)
                z2 = ln_apply(r3[:, 0:nw7, :], nw7, "l2")
                zg = tw.tile([128, TG, 128], BF, tag="t_zg")
                nc.vector.tensor_tensor(
                    zg[:, 0:nw7, :], z2[:, 0:nw7, :],
                    lng_s[:].unsqueeze(1).broadcast_to([128, nw7, 128]), MUL)
                og = tw.tile([128, TG, 128], FP, tag="t_og")
                nc.vector.tensor_tensor(
                    og[:, 0:nw7, :], zg[:, 0:nw7, :],
                    lnb_s[:].unsqueeze(1).broadcast_to([128, nw7, 128]), ADD)
                nc.sync.dma_start(out[:, cs], og[:, 0:nw7, :])

            msg_t = [None]
            closed_hi = []
            pending = [None]
            tails_done = set()

            def part_a(ws):
                """out-net first linear + silu (sigmoid table) for windows ws
                (consecutive), into h1_all."""
                wmin = min(ws)
                nw = len(ws)
                p1 = pop.tile([128, 512], FP, tag="p1T")
                nc.tensor.matmul(p1[:, 0:nw * 128], w1o_s[:],
                                 msgT[:, wmin:wmin + nw, :],
                                 start=True, stop=True)
                sg = wk.tile([128, 512], BF, tag="a_sg")
                nc.scalar.activation(sg[:, 0:nw * 128], p1[:, 0:nw * 128],
                                     SIGM, bias=b1o_s[:, 0:1])
                ts_ = wk.tile([128, 512], BF, tag="a_ts")
                nc.scalar.activation(ts_[:, 0:nw * 128], p1[:, 0:nw * 128],
                                     IDEN, bias=b1o_s[:, 0:1])
                h1 = wk.tile([128, 512], BF, tag="a_h1")
                nc.vector.tensor_tensor(h1[:, 0:nw * 128], ts_[:, 0:nw * 128],
                                        sg[:, 0:nw * 128], MUL)
                for k, w in enumerate(sorted(ws)):
                    pt = pop.tile([128, 128], BF, tag="a_pt")
                    nc.tensor.transpose(pt[:], h1[:, k * 128:(k + 1) * 128],
                                        id_s[:])
                    nc.scalar.activation(h1_all[:, w, :], pt[:], COPY)
                # hi windows close descending, so every tail group whose
                # window range is fully closed can run now — hidden under
                # the remaining gather wall
                for g0 in range(0, NW, TG):
                    if g0 >= wmin and g0 not in tails_done:
                        tails_done.add(g0)
                        tail_group(g0)

            def scatter_group(gstate):
                """Emit scatter matmuls + window closes for a finished chunk."""
                ci, sel_tiles = gstate
                ctile, coff = ck_tiles[ci]
                for mi, sel in sel_tiles:
                    for t in range(8):
                        gt = mi * 8 + t
                        p, w, first, last = tiles[gt]
                        loc = (gt * 128 - coff) // 128
                        if first:
                            msg_t[0] = pmp.tile([128, 128], FP, tag="msgw",
                                                name="msgw")
                        nc.tensor.matmul(msg_t[0][:],
                                         ctile[:, loc, 128:256],
                                         sel[:, t, :],
                                         start=first, stop=last)
                        if last:
                            if p == 0:
                                nc.scalar.activation(msgT[:, w, :],
                                                     msg_t[0][:], COPY)
                            else:
                                nc.vector.tensor_tensor(
                                    msgT[:, w, :], msg_t[0][:],
                                    msgT[:, w, :], ADD)
                                closed_hi.append(w)
                                if len(closed_hi) == 4 or w == 0:
                                    part_a(list(closed_hi))
                                    closed_hi.clear()

            for ci, (off, gsz, p) in enumerate(chunks):
                ctile, coff = ck_tiles[ci]
                # per-chunk row-side streams (full preload would blow SBUF)
                lrch = wk.tile([1, CHUNK], BF, tag="lrch")
                nc.sync.dma_start(lrch[0:1, 0:gsz], lrowr[0:1, off:off + gsz])
                each = wk.tile([4, CHUNK // 4], BF, tag="each")
                nc.sync.dma_start(each[:, 0:gsz // 4],
                                  ea4t[:, off // 4:(off + gsz) // 4])
                # scatter the PREVIOUS chunk first: its inputs are ready, so
                # the in-order PE does useful work while this chunk's gather
                # transfer completes
                if pending[0] is not None:
                    scatter_group(pending[0])
                    pending[0] = None
                gmacs = list(range(off // MACRO, (off + gsz) // MACRO))
                strip = wk.tile([128, 32], FP, tag="strip")
                for k, m in enumerate(gmacs):
                    moff = m * MACRO            # stream offset
                    # one-hot [s, e] for A-expand
                    prepb = wk.tile([128, 1024], BF, tag="prepb")
                    lmoff = moff - off
                    for h in range(2):
                        prep = ppp.tile([128, 512], FP, tag="prep")
                        nc.tensor.matmul(prep[:], ones_s[:],
                                         lrch[0:1, lmoff + h * 512:
                                              lmoff + (h + 1) * 512],
                                         start=True, stop=True)
                        nc.scalar.activation(prepb[:, h * 512:(h + 1) * 512],
                                             prep[:], COPY)
                    oht = wk.tile([128, 1024], BF, tag="oht")
                    nc.vector.tensor_tensor(
                        oht[:], prepb[:],
                        iocb_s[:, 0:1].broadcast_to([128, 1024]), EQ)
                    silu4 = wk.tile([128, 1024], BF, tag="silu4")
                    for h in range(2):
                        hid = php.tile([128, 512], FP, tag="hid")
                        # bank-wide opener first: start=True clears has_written
                        # for the WHOLE bank, so it must come before the
                        # per-region accumulating matmuls
                        m512 = lmoff // 512 + h
                        nc.tensor.matmul(hid[:],
                                         each[:, m512 * 128:(m512 + 1) * 128],
                                         we4_s[:], start=True, stop=False)
                        for t in range(4):
                            gt = m * 8 + h * 4 + t
                            w = tiles[gt][1]
                            nc.tensor.matmul(
                                hid[:, t * 128:(t + 1) * 128],
                                oht[:, (h * 4 + t) * 128:(h * 4 + t + 1) * 128],
                                aloc_s[:, w * 128:(w + 1) * 128],
                                start=False, stop=False)
                        loc0 = (moff - coff) // 128 + h * 4
                        nc.tensor.matmul(hid[:], id_s[:],
                                         ctile[:, loc0:loc0 + 4, 0:128],
                                         start=False, stop=True)
                        nc.scalar.activation(silu4[:, h * 512:(h + 1) * 512],
                                             hid[:], SILU)
                    sw = wk.tile([128, 8, 128], BF, tag="sw")
                    nc.vector.tensor_tensor(
                        sw[:],
                        silu4[:].rearrange("p (a b) -> p a b", b=128),
                        w2r_s[:].unsqueeze(1).broadcast_to([128, 8, 128]), MUL)
                    nc.vector.tensor_reduce(strip[:, k * 8:(k + 1) * 8],
                                            sw[:], axis=AXX, op=ADD)
                # strip -> att (one table flip per chunk)
                gt0 = gmacs[0] * 8
                ntl = len(gmacs) * 8
                stb = wk.tile([128, 32], FP, tag="stb")
                nc.vector.tensor_tensor(stb[:, 0:ntl], strip[:, 0:ntl],
                                        mb_s[:, gt0:gt0 + ntl], ADD)
                att = wk.tile([128, 32], BF, tag="att")
                nc.scalar.activation(att[:, 0:ntl], stb[:, 0:ntl], SIGM)
                # sel one-hots (att folded) for each macro
                sel_tiles = []
                for k, m in enumerate(gmacs):
                    se = wk.tile([128, 8, 128], BF, tag="selEQ")
                    nc.vector.tensor_tensor(
                        se[:],
                        iof_s[:].unsqueeze(1).broadcast_to([128, 8, 128]),
                        lrc_s[:, m * 8:(m + 1) * 8].unsqueeze(2)
                        .broadcast_to([128, 8, 128]), EQ)
                    sa = selp.tile([128, 8, 128], BF, tag="selA")
                    nc.vector.tensor_tensor(
                        sa[:], se[:],
                        att[:, k * 8:(k + 1) * 8].unsqueeze(2)
                        .broadcast_to([128, 8, 128]), MUL)
                    sel_tiles.append((m, sa))
                pending[0] = (ci, sel_tiles)
                # prefetch AFTER the previous chunk's readers are emitted so
                # buffer reuse is dependency-tracked
                if ci + 2 < len(chunks):
                    issue_gather(ci + 2)
            scatter_group(pending[0])

            for _p in (tw, tp, pop, pmp, php, ppp, selp, idp, chp):
                _p.release()

    nc.compile()
    return nc


def _phase2_inputs(plan, streams, p1_results, inp, slot_node):
    att_w1, att_w2 = inp["att_w1"], inp["att_w2"]

    def unwrap(a, c):
        a = np.asarray(a, dtype=F32)
        return a.reshape(128, NW, 128).transpose(1, 0, 2).reshape(NLP, 128)

    # merged [B ; xm] table in original node order
    tab = np.zeros((N, 256), F32)
    for c in range(NCORES):
        bl = unwrap(p1_results[c]["b_out"], c)
        xl = unwrap(p1_results[c]["xm_out"], c)
        sn = slot_node[c]
        m = sn >= 0
        tab[sn[m], 0:128] = bl[m]
        tab[sn[m], 128:256] = xl[m]
    tab = _bf(tab)

    w2op = inp["out_ln_g"][:, None] * inp["out_w2"]
    b2op = inp["out_ln_b"] @ inp["out_w2"] + inp["out_b2"]
    we = att_w1[2 * D:2 * D + 1].ravel()       # [128]
    we4 = np.zeros((4, 512), F32)
    for k in range(4):
        we4[k, k * 128:(k + 1) * 128] = we

    consts = {
        "tab_lo": tab[0:SPLIT], "tab_hi": tab[SPLIT:N],
        "iof": _bf(np.broadcast_to(np.arange(128)[None, :], (128, 128))),
        "iota_col": _f32(np.arange(128)[:, None]),
        "iota_colb": _bf(np.arange(128)[:, None]),
        "ones1": _bf(np.ones((1, 128))), "ident": _bf(np.eye(128)),
        "we4blk": _bf(we4),
        "w2r": _bf(np.broadcast_to(att_w2[:, 0][None, :], (128, 128))),
        "w1o": _bf(inp["out_w1"]), "b1o_col": _f32(inp["out_b1"][:, None]),
        "w2op": _bf(w2op), "b2op_row": _bf(b2op[None, :]),
        "lng_rep": _bf(np.broadcast_to(inp["ln_g"][None, :], (128, 128))),
        "lnb_rep": _bf(np.broadcast_to(inp["ln_b"][None, :], (128, 128))),
        "eps_col": _f32(np.full((128, 1), LN_EPS)),
    }
    in_maps = []
    for c in range(NCORES):
        in_maps.append({
            "aloc": np.ascontiguousarray(np.asarray(p1_results[c]["a_out"])),
            "xres": np.ascontiguousarray(np.asarray(p1_results[c]["x_out"])),
            "idx": streams["idx"][c],
            "lrow_row": streams["lrow_row"][c],
            "lrowc": streams["lrowc"][c],
            "mb": streams["mb"][c],
            "ea4": streams["ea4"][c],
            **consts,
        })
    return in_maps


def kernel(**inputs):
    inp = {k: np.asarray(v) for k, v in inputs.items()}
    b2 = float(np.asarray(inp["att_b2"]).ravel()[0])
    plan, streams, slot_node = _prep_edges(
        inp["row"], inp["col"], inp["edge_attr"], inp["edge_mask"], b2)

    # phase-1 input: h gathered into (core, slot) layout
    h = _f32(inp["h"])
    h_slots = np.zeros((NCORES, NLP, D), F32)
    for c in range(NCORES):
        sn = slot_node[c]
        m = sn >= 0
        h_slots[c][m] = h[sn[m]]

    p1 = _run_phase1(h_slots, inp)

    key = (plan["sizes_lo"], plan["sizes_hi"])
    if _cache.get("p2_key") != key:
        _cache["p2"] = _build_phase2(plan)
        _cache["p2_key"] = key
    nc2 = _cache["p2"]
    in_maps = _phase2_inputs(plan, streams, p1, inp, slot_node)
    res = run_bass_kernel_spmd(nc2, in_maps, core_ids=list(range(NCORES)),
                               trace=TRACE)
    LAST_RESULTS["phase2_results"] = res.results
    if TRACE:
        LAST_RESULTS["phase2_ns"] = res.exec_time_ns
        LAST_RESULTS["phase2_res"] = res

    out = np.zeros((N, D), F32)
    for c in range(NCORES):
        o = np.asarray(res.results[c]["out"], F32)
        o = o.reshape(128, NW, 128).transpose(1, 0, 2).reshape(NLP, 128)
        sn = slot_node[c]
        m = sn >= 0
        out[sn[m]] = o[m]
    return out.astype(F32)
